# revision 32
# baseline (speedup 1.0000x reference)
"""Trainium2 Bass kernel for FeatureAugmentationNetwork2.

Reference computation (N=M=8192, H=512, tau=1, c=0.5):
    q = features @ Wq.T + bq
    k = memory_features @ Wk.T + bk
    attn = softmax(q @ k.T, axis=-1)
    out = c * features + (1-c) * attn @ memory_features

Sharding: features (queries) split across 8 cores on the N axis;
memory_features / weights replicated.  Each core computes its
[1024, 8192] attention slab independently; outputs are concatenated.

Algebraic restructuring (exact):
  - bk adds a per-row constant to the logits -> softmax-invariant -> dropped.
  - S = q @ k.T = (features @ W2 + b2) @ memory.T
    with W2 = Wq.T @ Wk (computed on-chip), b2 = bq @ Wk.
  - softmax without a row max: exp(s - C) with fixed C = 100 into bf16 E
    tiles (logits ~N(0,512); all row maxes land in bf16 range with huge
    margin).
  - The [m, n]-layout exp tile feeds attn.V as lhsT without any transpose;
    the softmax denominator is fused into the AV matmul via a constant
    column appended to V.

fp8 AV path (the big win vs the bf16 version):
  - The AV matmul runs in fp8 (e4m3) with MatmulPerfMode.DoubleRow: each
    matmul instruction contracts over TWO 128-row memory tiles (256 deep)
    at double rate, halving AV's PE time.
  - V is pre-quantized on the host: V8 = e4m3(32*V) with the denominator
    column = 32.0 (the common factor 32 cancels in aug/den).
  - E must be squeezed into e4m3's ~12-nat dynamic window, which requires
    a per-QUERY scale F[n].  Any per-query scale cancels exactly in the
    aug/den ratio, so correctness only needs range-fit, not exactness.
    F[n] = T0 / rowmax_round(E), flash-style across rounds:
    F <- min(F, T0/rowmax_g); the SBUF aug accumulator (including its
    denominator column) is rescaled by F_new/F_old before adding round g.
  - rowmax per round: DVE running elementwise max over the 16 E tiles
    ([m,n] layout), then 8 PE transposes + a DVE X-reduce to get the
    cross-partition max into per-query layout [128, 8].
  - The broadcast of F along the m axis (for the E*F quantize) is a
    1-contraction PE matmul: ones[1,128].T @ f_row[1,512] -> PSUM, copied
    to SBUF bf16; quantize = DVE tensor_tensor mult with fp8 output.

Precision: S path in fp16 as before (logit err ~0.009); E quantization
cancels in aug/den; the V e4m3 mantissa (2.4% rms) dominates the error:
predicted end-to-end rel err 1.84e-2 vs the 2e-2 gate (CPU sim,
calibrated to 0.3% against the bf16 kernel's measured 2.14e-3).

Schedule: stats/rescale/AV for round g-1 are emitted interleaved with
round g's S phase, so the PE never waits on the DVE stats chain: the
PE stream is [S_g | transposes_{g-1} | bcast_{g-1} | AV_{g-1} | S_{g+1}...]
with the DVE work (max tracking, stats, quantize) hidden under S.
"""

from contextlib import ExitStack

import ml_dtypes
import numpy as np

import concourse.bass as bass
import concourse.tile as tile
from concourse import bacc, mybir
from concourse.alu_op_type import AluOpType
from concourse.bass_utils import run_bass_kernel_spmd
from concourse.masks import make_identity

N_CORES = 8
N, M, H = 8192, 8192, 512
N_LOC = N // N_CORES  # 1024 query rows per core
C_OFF = 100.0  # fixed softmax exp offset
MERGE = 0.5
VSCALE = 32.0  # host-side V/denominator scale (cancels in aug/den)
T0 = 120.0  # target scaled row max for fp8 E (<240 with slack)

F32 = mybir.dt.float32
F16 = mybir.dt.float16
BF16 = mybir.dt.bfloat16
F8 = mybir.dt.float8e4

HH = H // 2  # 256
VW8 = H + 8  # fp8 V row: [512 V | 32.0 | 7 pad]

NT = N_LOC // 128  # 8  query-row tiles
MT = M // 128  # 64 memory-row tiles
HC = H // 128  # 4  feature-dim chunks
GROUP = 16  # memory tiles per AV accumulation round
NPAIR = GROUP // 2  # 8  DoubleRow pairs per round
MC = 8  # memory tiles per memT chunk (2 chunks per round)
NH = N_LOC // 512  # 2  n halves (512-wide matmul free dim)
# round partition of the 64 memory tiles (8-tile tail rounds measured SLOWER:
# their S phases are too short to hide the per-round DVE chains)
ROUND_TILES = [(0, 16), (16, 16), (32, 16), (48, 16)]
N_ROUNDS = len(ROUND_TILES)


def _emit(nc, tc, ctx, d):
    main_sb = ctx.enter_context(tc.tile_pool(name="main_sb", bufs=1))
    bias_t = main_sb.tile([128, 1], F32)
    nc.vector.memset(bias_t[:], -C_OFF)
    aug = main_sb.tile([128, NT, H + 1], F32)  # col 512 holds the denominator
    rh = main_sb.tile([128, NT], F32)
    feat = main_sb.tile([128, NT, H], F16)  # holds MERGE * features (host-scaled)
    q2T = main_sb.tile([128, HC, N_LOC], F16)
    # identity for PE transposes (bf16 inputs only)
    ident = main_sb.tile([128, 128], BF16)
    make_identity(nc, ident[:])
    ones_row = main_sb.tile([1, 128], BF16)
    nc.vector.memset(ones_row[:], 1.0)
    # per-query flash scale state, chunk-major [p, nt] (query n = nt*128+p)
    Fprev = main_sb.tile([128, NT], F32)
    Finv = main_sb.tile([128, NT], F32)
    ratio = main_sb.tile([128, NT], F32)
    f_bf = main_sb.tile([128, NT], BF16)
    rm = main_sb.tile([128, NT], F32)

    mv_pool = ctx.enter_context(tc.tile_pool(name="mv", bufs=2))
    met_pool = ctx.enter_context(tc.tile_pool(name="met", bufs=4))
    s_ps_pool = ctx.enter_context(tc.tile_pool(name="sps", bufs=3, space="PSUM"))
    av1_pool = ctx.enter_context(tc.tile_pool(name="av1", bufs=2, space="PSUM"))
    av2_pool = ctx.enter_context(tc.tile_pool(name="av2", bufs=2, space="PSUM"))
    stat_ps = ctx.enter_context(tc.tile_pool(name="stat", bufs=1, space="PSUM"))
    et_pool = ctx.enter_context(tc.tile_pool(name="et", bufs=2 * NPAIR - 2))
    e8_pool = ctx.enter_context(tc.tile_pool(name="e8", bufs=NPAIR + 1))
    rmax_pool = ctx.enter_context(tc.tile_pool(name="rmax", bufs=2))
    fb_pool = ctx.enter_context(tc.tile_pool(name="fb", bufs=2))
    out_pool = ctx.enter_context(tc.tile_pool(name="out_sb", bufs=2))

    def load_mv(start, ntiles):
        """Natural-layout fp8 memory tiles [V*32 | 32] for the AV matmul."""
        mv_t = mv_pool.tile([128, GROUP, VW8], F8, tag="mv")
        for half in range(ntiles // MC):
            base = (start + half * MC) * 128
            nc.sync.dma_start(
                mv_t[:, half * MC : (half + 1) * MC, :],
                d["memv8"][base : base + MC * 128, :].rearrange(
                    "(t p) h -> p t h", p=128
                ),
            )
        return mv_t

    # memT is host-transposed ([H, M] fp16), so the lhsT tiles arrive via
    # plain strided DMAs instead of exclusive-engine XBAR transposes.
    memT_r = d["memT"].rearrange("(c p) m -> p c m", p=128)

    def load_met(start, ntiles):
        """fp16 memT chunks (8 memory tiles each) from the host-transposed array."""
        mets = []
        for c in range(ntiles // MC):
            base = (start + c * MC) * 128
            met8 = met_pool.tile([128, HC, MC * 128], F16, tag="met")
            nc.sync.dma_start(met8[:], memT_r[:, :, base : base + MC * 128])
            mets.append(met8)
        return mets

    # ------------------------------ preamble -------------------------------
    with ExitStack() as pre_ctx:
        pre_w = pre_ctx.enter_context(tc.tile_pool(name="pre_w", bufs=1))
        # wqh/bqh ride the scalar engine's HWDGE queue so both weight loads
        # issue in parallel during framework boot.  Host pre-arranges the
        # weights into the on-chip [p, c, h] layout for contiguous loads.
        wqh = pre_w.tile([128, HC, H], F16)
        wkh = pre_w.tile([128, HC, H], F16)
        # split loads so the first W2 matmul (needs only oc-chunk 0) can
        # start as soon as the first halves land
        wq_r = d["wqh"].rearrange("p (c h) -> p c h", c=HC)
        wk_r = d["wkh"].rearrange("p (c h) -> p c h", c=HC)
        for c in range(HC):
            nc.scalar.dma_start(wqh[:, c : c + 1, :], wq_r[:, c : c + 1, :])
            nc.sync.dma_start(wkh[:, c : c + 1, :], wk_r[:, c : c + 1, :])
        bqh = pre_w.tile([128, HC], F16)
        nc.scalar.dma_start(bqh[:], d["bqh"].rearrange("p c -> p c"))
        featT = pre_w.tile([128, HC, N_LOC], F16)
        nc.sync.dma_start(
            featT[:], d["featT"].rearrange("(c p) n -> p c n", p=128)
        )
        mets0 = []
        for c in range(4):
            # round-0 chunks share the steady-state "met" tag slots (the
            # round-1 prefetches then rotate in as round 0's are consumed)
            met4 = met_pool.tile([128, HC, 4 * 128], F16, tag="met", name=f"met4_{c}")
            nc.sync.dma_start(
                met4[:], memT_r[:, :, c * 4 * 128 : (c + 1) * 4 * 128]
            )
            mets0.append(met4)
        mets = mets0
        mv_t = load_mv(0, GROUP)

        # W2[i, j] = sum_o Wq[o, i] * Wk[o, j]   (fp16 matmul, fp16 result)
        w2r = pre_w.tile([128, HC, H], F16)
        for ic in range(HC):
            ps = s_ps_pool.tile([128, H], F32, tag="sps", name=f"w2ps{ic}")
            for oc in range(HC):
                nc.tensor.matmul(
                    ps[:],
                    wqh[:, oc, ic * 128 : (ic + 1) * 128],
                    wkh[:, oc, :],
                    start=(oc == 0),
                    stop=(oc == HC - 1),
                )
            nc.vector.tensor_copy(w2r[:, ic, :], ps[:])

        # b2T[j] = sum_o Wk[o, j] * bq[o]
        b2full = s_ps_pool.tile([128, H], F32, tag="sps", name="b2ps")
        b2ps = b2full[:, :HC]
        for jc in range(HC):
            for oc in range(HC):
                nc.tensor.matmul(
                    b2ps[:, jc : jc + 1],
                    wkh[:, oc, jc * 128 : (jc + 1) * 128],
                    bqh[:, oc : oc + 1],
                    start=(oc == 0),
                    stop=(oc == HC - 1),
                    skip_group_check=True,
                )
        b2t = pre_w.tile([128, HC], F32)
        nc.vector.tensor_copy(b2t[:], b2ps)

        # q2T[j, n] = sum_i W2[i, j] featT[i, n] + b2T[j]   (fp16 matmul);
        # nh-major so the n-half the first S tiles consume is ready early.
        for nh in range(NH):
            for jc in range(HC):
                ps = s_ps_pool.tile([128, 512], F32, tag="sps", name=f"q2ps{jc}_{nh}")
                for ic in range(HC):
                    nc.tensor.matmul(
                        ps[:],
                        w2r[:, ic, jc * 128 : (jc + 1) * 128],
                        featT[:, ic, nh * 512 : (nh + 1) * 512],
                        start=(ic == 0),
                        stop=(ic == HC - 1),
                    )
                nc.vector.tensor_scalar_add(
                    q2T[:, jc, nh * 512 : (nh + 1) * 512], ps[:], b2t[:, jc : jc + 1]
                )
        pre_ctx.close()  # release wqh/wkh/bqh/w2r/b2t/featT

    # ---------------- main loop over memory-tile rounds --------------------
    # Round r state carried into round r+1's emission window:
    state = {}  # r -> (ets, e8s, mv, rmax)

    def emit_transposes(r):
        """8 PE transposes of rmax_r -> trmax PSUM, feeding the X-reduce."""
        rmax_t = state[r]["rmax"]
        trmax = stat_ps.tile([128, NT, 128], BF16, tag="stat", name=f"trmax{r}")
        for c in range(NT):
            nc.tensor.transpose(
                trmax[:, c, :], rmax_t[:, c * 128 : (c + 1) * 128], ident[:]
            )
        state[r]["trmax"] = trmax

    def emit_stats_dve(r):
        """Per-query scale update for round r (all [128, NT] pp-layout)."""
        nc.vector.tensor_reduce(
            rm[:], state[r]["trmax"][:], mybir.AxisListType.X, AluOpType.max
        )
        # raw = T0 / rowmax; Fnew = min(Fprev, raw); ratio = Fnew * Finv_old
        nc.vector.reciprocal(rm[:], rm[:])
        if r == 0:
            nc.vector.tensor_scalar_mul(Fprev[:], rm[:], T0)
        else:
            nc.vector.tensor_scalar_mul(rm[:], rm[:], T0)
            nc.vector.tensor_tensor(Fprev[:], Fprev[:], rm[:], AluOpType.min)
            nc.vector.tensor_tensor(ratio[:], Fprev[:], Finv[:], AluOpType.mult)
        nc.vector.reciprocal(Finv[:], Fprev[:])
        nc.vector.tensor_copy(f_bf[:], Fprev[:])

    def emit_frow_transposes(r):
        """f_bf [128, NT] -> frow PSUM [1, N_LOC] (row layout for bcast)."""
        frow = stat_ps.tile([1, N_LOC], BF16, tag="stat", name=f"frow{r}")
        for c in range(NT):
            nc.tensor.transpose(
                frow[:, c * 128 : (c + 1) * 128], f_bf[:, c : c + 1], ident[:]
            )
        state[r]["frow"] = frow

    def emit_frow_copy(r):
        frow_sb = fb_pool.tile([1, N_LOC], BF16, tag="frow_sb")
        nc.vector.tensor_copy(frow_sb[:], state[r]["frow"][:])
        state[r]["frow_sb"] = frow_sb

    def emit_bcast(r):
        """Fb2[p, i, n] = f[n] via 1-contraction matmuls + pair-replicated copy."""
        fb2 = fb_pool.tile([128, 2, N_LOC], BF16, tag="fb_sb")
        for nh in range(NH):
            fb_ps = stat_ps.tile([128, 512], F32, tag="stat", name=f"fbps{r}_{nh}")
            nc.tensor.matmul(
                fb_ps[:],
                ones_row[:],
                state[r]["frow_sb"][:, nh * 512 : (nh + 1) * 512],
                start=True,
                stop=True,
            )
            for i in range(2):
                nc.vector.tensor_copy(fb2[:, i, nh * 512 : (nh + 1) * 512], fb_ps[:])
        state[r]["fb2"] = fb2

    def emit_rescale(r):
        """e8 = e4m3(et * F[n]).

        Steady rounds: in-place bf16 multiply on DVE (fast path), then ONE
        gpsimd software-DGE DMA per pair does the bf16 -> fp8 cast (keeping
        the slow fp8-output store path off the vector engine; latency hides
        under the next round's S phase).
        Epilogue round: direct DVE fp8-out multiply per pair -- slower per
        element but lowest latency to first/last pair, which gates AV."""
        fb2 = state[r]["fb2"]
        epilogue = r == N_ROUNDS - 1
        npair = len(state[r]["ets"])
        e8s = [None] * npair
        # steady rounds: all pairs via mult + Pool-cast (latency hidden).
        # epilogue: back half via Pool-cast (emitted first so the casts
        # start early), front half DVE-direct -- both engines in parallel.
        if epilogue:
            # all-DVE-direct: deterministic ~2.3us/pair latency beats the
            # Pool path's sem+transfer cliff when nothing hides it
            order = [(p, False) for p in range(npair)]
        else:
            order = [(p, True) for p in range(npair)]
        for p, pool_path in order:
            et = state[r]["ets"][p]
            e8 = e8_pool.tile([128, 2, N_LOC], F8, tag="e8", name=f"e8_{r}_{p}")
            etf = et[:].rearrange("p a n -> p (a n)")
            fbf = fb2[:].rearrange("p a n -> p (a n)")
            e8f = e8[:].rearrange("p a n -> p (a n)")
            if pool_path:
                nc.vector.tensor_tensor(etf, etf, fbf, AluOpType.mult)
                nc.gpsimd.dma_start(e8f, etf)
            else:
                nc.vector.tensor_tensor(e8f, etf, fbf, AluOpType.mult)
            e8s[p] = e8
        state[r]["e8s"] = e8s

    def emit_av(r):
        """DoubleRow fp8 AV + denominator, flash fixup, merge/store on last."""
        e8s = state[r]["e8s"]
        mv8 = state[r]["mv"]
        npair = len(e8s)
        # accumulation order is free within a PSUM group; pairs become
        # available in emission order either way
        porder = list(range(npair))
        for nt in range(NT):
            av1 = av1_pool.tile([128, HH + 1], F32, tag="av1")
            av2 = av2_pool.tile([128, HH], F32, tag="av2")
            for k, p in enumerate(porder):
                eb = e8s[p][:, :, nt * 128 : (nt + 1) * 128]
                nc.tensor.matmul(
                    av2[:],
                    eb,
                    mv8[:, 2 * p : 2 * p + 2, 0:HH],
                    start=(k == 0),
                    stop=(k == npair - 1),
                    perf_mode=mybir.MatmulPerfMode.DoubleRow,
                )
                nc.tensor.matmul(
                    av1[:],
                    eb,
                    mv8[:, 2 * p : 2 * p + 2, HH : H + 1],
                    start=(k == 0),
                    stop=(k == npair - 1),
                    perf_mode=mybir.MatmulPerfMode.DoubleRow,
                )
            if r == 0:
                nc.vector.tensor_copy(aug[:, nt, 0:HH], av2[:])
                nc.vector.tensor_copy(aug[:, nt, HH : H + 1], av1[:])
            else:
                nc.vector.tensor_scalar_mul(
                    aug[:, nt, :], aug[:, nt, :], ratio[:, nt : nt + 1]
                )
                nc.vector.tensor_tensor(
                    aug[:, nt, 0:HH], aug[:, nt, 0:HH], av2[:], AluOpType.add
                )
                nc.vector.tensor_tensor(
                    aug[:, nt, HH : H + 1],
                    aug[:, nt, HH : H + 1],
                    av1[:],
                    AluOpType.add,
                )
            if r == N_ROUNDS - 1:
                # denominator complete for this nt: normalize + merge + store
                nc.vector.reciprocal(rh[:, nt : nt + 1], aug[:, nt, H : H + 1])
                nc.vector.tensor_scalar_mul(
                    rh[:, nt : nt + 1], rh[:, nt : nt + 1], 1.0 - MERGE
                )
                # feat already holds MERGE * features (host pre-scaled)
                o = out_pool.tile([128, H], F32, tag="out")
                nc.vector.scalar_tensor_tensor(
                    o[:],
                    aug[:, nt, 0:H],
                    rh[:, nt : nt + 1],
                    feat[:, nt, :],
                    op0=AluOpType.mult,
                    op1=AluOpType.add,
                )
                nc.sync.dma_start(d["out"][nt * 128 : (nt + 1) * 128, :], o[:])

    def emit_s_tiles(g, start, ntiles, tl_range, ets, rmax_t):
        csz = ntiles // len(mets)
        for tl in tl_range:
            met8 = mets[tl // csz]
            t = tl % csz
            pi, i = tl // 2, tl % 2
            if i == 0:
                ets.append(
                    et_pool.tile([128, 2, N_LOC], BF16, tag="et", name=f"et{g}_{pi}")
                )
            et = ets[pi]
            for nh in range(NH):
                sp = s_ps_pool.tile([128, 512], F32, tag="sps")
                for jc in range(HC):
                    nc.tensor.matmul(
                        sp[:],
                        met8[:, jc, t * 128 : (t + 1) * 128],
                        q2T[:, jc, nh * 512 : (nh + 1) * 512],
                        start=(jc == 0),
                        stop=(jc == HC - 1),
                    )
                nc.scalar.activation(
                    et[:, i, nh * 512 : (nh + 1) * 512],
                    sp[:],
                    mybir.ActivationFunctionType.Exp,
                    bias=bias_t[:],
                )
            # running elementwise max for this round's row-max
            if tl == 0:
                nc.vector.tensor_copy(rmax_t[:], et[:, 0, :])
            else:
                nc.vector.tensor_tensor(rmax_t[:], rmax_t[:], et[:, i, :], AluOpType.max)

    for g, (start, ntiles) in enumerate(ROUND_TILES):
        if g + 1 < N_ROUNDS:
            next_mets = load_met(*ROUND_TILES[g + 1])
            next_mv = load_mv(*ROUND_TILES[g + 1])
        if g == 3:
            # merge-side features (fp16, pre-scaled by MERGE) load late
            nc.sync.dma_start(
                feat[:], d["featm"].rearrange("(t p) h -> p t h", p=128)
            )
        r = g - 1  # round whose stats/AV are interleaved with S_g
        ets = []
        rmax_t = rmax_pool.tile([128, N_LOC], BF16, tag="rmax", name=f"rmax{g}")
        state[g] = {"rmax": rmax_t, "mv": mv_t, "ets": ets}

        # The whole stats->rescale chain sits at the TOP of the round: the
        # AV of round r-1 (end of the previous emission) absorbs round r's
        # exp/max tail, so transp8_r starts unstalled, and the rescale's
        # DVE mults run ahead of this round's max ops in the DVE queue --
        # the Pool casts then complete long before AV_r needs the pairs.
        # Round 1 has no preceding AV on the PE to absorb round 0's exp/max
        # tail, so lead with one S tile before the transposes there.
        lead = 1 if r == 0 else 0
        if lead:
            emit_s_tiles(g, start, ntiles, range(0, 1), ets, rmax_t)
        if r >= 0:
            emit_transposes(r)
            emit_stats_dve(r)
        emit_s_tiles(g, start, ntiles, range(lead, lead + 1), ets, rmax_t)
        if r >= 0:
            emit_frow_transposes(r)
            emit_frow_copy(r)
        emit_s_tiles(g, start, ntiles, range(lead + 1, lead + 2), ets, rmax_t)
        if r >= 0:
            emit_bcast(r)
            emit_rescale(r)
        emit_s_tiles(g, start, ntiles, range(lead + 2, ntiles), ets, rmax_t)
        if r >= 0:
            emit_av(r)
            del state[r]
        if g + 1 < N_ROUNDS:
            mets = next_mets
            mv_t = next_mv

    # epilogue: stats + AV for the last round
    r = N_ROUNDS - 1
    emit_transposes(r)
    emit_stats_dve(r)
    emit_frow_transposes(r)
    emit_frow_copy(r)
    emit_bcast(r)
    emit_rescale(r)
    emit_av(r)


def build_module():
    nc = bacc.Bacc("TRN2", target_bir_lowering=False, debug=False)
    d = {
        "featm": nc.dram_tensor("featm", [N_LOC, H], F16, kind="ExternalInput").ap(),
        "featT": nc.dram_tensor("featT", [H, N_LOC], F16, kind="ExternalInput").ap(),
        "memT": nc.dram_tensor("memT", [H, M], F16, kind="ExternalInput").ap(),
        "memv8": nc.dram_tensor("memv8", [M, VW8], F8, kind="ExternalInput").ap(),
        "wqh": nc.dram_tensor("wqh", [128, H // 128 * H], F16, kind="ExternalInput").ap(),
        "wkh": nc.dram_tensor("wkh", [128, H // 128 * H], F16, kind="ExternalInput").ap(),
        "bqh": nc.dram_tensor("bqh", [128, H // 128], F16, kind="ExternalInput").ap(),
        "out": nc.dram_tensor("out", [N_LOC, H], F32, kind="ExternalOutput").ap(),
    }
    with tile.TileContext(nc) as tc, ExitStack() as ctx:
        _emit(nc, tc, ctx, d)
    nc.compile()
    return nc


_CACHED = None


def _warrange(w):  # [512, 512] -> on-chip [p, c*h] layout, contiguous DMA
    w16 = np.asarray(w, dtype=np.float32).astype(np.float16)
    return np.ascontiguousarray(
        w16.reshape(H // 128, 128, H).transpose(1, 0, 2).reshape(128, -1)
    )


def _mem8(mem):  # [M, H] f32 -> [M, VW8] e4m3 of [32*V | 32 | pad]
    out = np.zeros((M, VW8), dtype=ml_dtypes.float8_e4m3)
    scaled = np.clip(mem * VSCALE, -240.0, 240.0)
    out[:, :H] = scaled.astype(ml_dtypes.float8_e4m3)
    out[:, H] = np.float32(VSCALE)
    return out


def kernel(features, memory_features, Wq, bq, Wk, bk=None, **_ignored):
    global _CACHED
    if _CACHED is None:
        _CACHED = build_module()
    nc = _CACHED

    features = np.ascontiguousarray(np.asarray(features, dtype=np.float32))
    memory_features = np.ascontiguousarray(np.asarray(memory_features, dtype=np.float32))
    memT = np.ascontiguousarray(memory_features.T.astype(np.float16))
    memv8 = _mem8(memory_features)
    feath = features.astype(np.float16)  # sharded then transposed per core
    featm = (MERGE * features).astype(np.float16)  # merge-side, pre-scaled
    wqh = _warrange(Wq)
    wkh = _warrange(Wk)
    bqh = np.ascontiguousarray(
        np.asarray(bq, dtype=np.float32).astype(np.float16).reshape(H // 128, 128).T
    )

    in_maps = []
    for c in range(N_CORES):
        in_maps.append(
            {
                "featm": featm[c * N_LOC : (c + 1) * N_LOC],
                "featT": np.ascontiguousarray(
                    feath[c * N_LOC : (c + 1) * N_LOC].T
                ),
                "memT": memT,
                "memv8": memv8,
                "wqh": wqh,
                "wkh": wkh,
                "bqh": bqh,
            }
        )
    res = run_bass_kernel_spmd(nc, in_maps, core_ids=list(range(N_CORES)))
    return np.concatenate([res.results[c]["out"] for c in range(N_CORES)], axis=0)


# revision 34
# speedup vs baseline: 1.0225x; 1.0225x over previous
"""Trainium2 Bass kernel for FeatureAugmentationNetwork2.

Reference computation (N=M=8192, H=512, tau=1, c=0.5):
    q = features @ Wq.T + bq
    k = memory_features @ Wk.T + bk
    attn = softmax(q @ k.T, axis=-1)
    out = c * features + (1-c) * attn @ memory_features

Sharding: features (queries) split across 8 cores on the N axis;
memory_features / weights replicated.  Each core computes its
[1024, 8192] attention slab independently; outputs are concatenated.

Algebraic restructuring (exact):
  - bk adds a per-row constant to the logits -> softmax-invariant -> dropped.
  - S = q @ k.T = (features @ W2 + b2) @ memory.T
    with W2 = Wq.T @ Wk (computed on-chip), b2 = bq @ Wk.
  - softmax without a row max: exp(s - C) with fixed C = 100 into bf16 E
    tiles (logits ~N(0,512); all row maxes land in bf16 range with huge
    margin).
  - The [m, n]-layout exp tile feeds attn.V as lhsT without any transpose;
    the softmax denominator is fused into the AV matmul via a constant
    column appended to V.

fp8 AV path (the big win vs the bf16 version):
  - The AV matmul runs in fp8 (e4m3) with MatmulPerfMode.DoubleRow: each
    matmul instruction contracts over TWO 128-row memory tiles (256 deep)
    at double rate, halving AV's PE time.
  - V is pre-quantized on the host: V8 = e4m3(32*V) with the denominator
    column = 32.0 (the common factor 32 cancels in aug/den).
  - E must be squeezed into e4m3's ~12-nat dynamic window, which requires
    a per-QUERY scale F[n].  Any per-query scale cancels exactly in the
    aug/den ratio, so correctness only needs range-fit, not exactness.
    F[n] = T0 / rowmax_round(E), flash-style across rounds:
    F <- min(F, T0/rowmax_g); the SBUF aug accumulator (including its
    denominator column) is rescaled by F_new/F_old before adding round g.
  - rowmax per round: DVE running elementwise max over the 16 E tiles
    ([m,n] layout), then 8 PE transposes + a DVE X-reduce to get the
    cross-partition max into per-query layout [128, 8].
  - The broadcast of F along the m axis (for the E*F quantize) is a
    1-contraction PE matmul: ones[1,128].T @ f_row[1,512] -> PSUM, copied
    to SBUF bf16; quantize = DVE tensor_tensor mult with fp8 output.

Precision: S path in fp16 as before (logit err ~0.009); E quantization
cancels in aug/den; the V e4m3 mantissa (2.4% rms) dominates the error:
predicted end-to-end rel err 1.84e-2 vs the 2e-2 gate (CPU sim,
calibrated to 0.3% against the bf16 kernel's measured 2.14e-3).

Schedule: stats/rescale/AV for round g-1 are emitted interleaved with
round g's S phase, so the PE never waits on the DVE stats chain: the
PE stream is [S_g | transposes_{g-1} | bcast_{g-1} | AV_{g-1} | S_{g+1}...]
with the DVE work (max tracking, stats, quantize) hidden under S.
"""

from contextlib import ExitStack

import ml_dtypes
import numpy as np

import concourse.bass as bass
import concourse.tile as tile
from concourse import bacc, mybir
from concourse.alu_op_type import AluOpType
from concourse.bass_utils import run_bass_kernel_spmd
from concourse.masks import make_identity

N_CORES = 8
N, M, H = 8192, 8192, 512
N_LOC = N // N_CORES  # 1024 query rows per core
C_OFF = 100.0  # fixed softmax exp offset
MERGE = 0.5
VSCALE = 32.0  # host-side V/denominator scale (cancels in aug/den)
T0 = 120.0  # target scaled row max for fp8 E (<240 with slack)

F32 = mybir.dt.float32
F16 = mybir.dt.float16
BF16 = mybir.dt.bfloat16
F8 = mybir.dt.float8e4

HH = H // 2  # 256
VW8 = H + 8  # fp8 V row: [512 V | 32.0 | 7 pad]

NT = N_LOC // 128  # 8  query-row tiles
MT = M // 128  # 64 memory-row tiles
HC = H // 128  # 4  feature-dim chunks
GROUP = 16  # memory tiles per AV accumulation round
NPAIR = GROUP // 2  # 8  DoubleRow pairs per round
MC = 8  # memory tiles per memT chunk (2 chunks per round)
NH = N_LOC // 512  # 2  n halves (512-wide matmul free dim)
# round partition of the 64 memory tiles (8-tile tail rounds measured SLOWER:
# their S phases are too short to hide the per-round DVE chains)
ROUND_TILES = [(0, 16), (16, 16), (32, 16), (48, 16)]
N_ROUNDS = len(ROUND_TILES)


def _emit(nc, tc, ctx, d):
    main_sb = ctx.enter_context(tc.tile_pool(name="main_sb", bufs=1))
    bias_t = main_sb.tile([128, 1], F32)
    nc.vector.memset(bias_t[:], -C_OFF)
    aug = main_sb.tile([128, NT, H + 1], F32)  # col 512 holds the denominator
    rh = main_sb.tile([128, NT], F32)
    feat = main_sb.tile([128, NT, H], F16)  # holds MERGE * features (host-scaled)
    q2T = main_sb.tile([128, HC, N_LOC], F16)
    # identity for PE transposes (bf16 inputs only)
    ident = main_sb.tile([128, 128], BF16)
    make_identity(nc, ident[:])
    ones_row = main_sb.tile([1, 128], BF16)
    nc.vector.memset(ones_row[:], 1.0)
    # per-query flash scale state, chunk-major [p, nt] (query n = nt*128+p)
    Fprev = main_sb.tile([128, NT], F32)
    Finv = main_sb.tile([128, NT], F32)
    ratio = main_sb.tile([128, NT], F32)
    f_bf = main_sb.tile([128, NT], BF16)
    rm = main_sb.tile([128, NT], F32)

    mv_pool = ctx.enter_context(tc.tile_pool(name="mv", bufs=2))
    met_pool = ctx.enter_context(tc.tile_pool(name="met", bufs=4))
    s_ps_pool = ctx.enter_context(tc.tile_pool(name="sps", bufs=3, space="PSUM"))
    av1_pool = ctx.enter_context(tc.tile_pool(name="av1", bufs=2, space="PSUM"))
    av2_pool = ctx.enter_context(tc.tile_pool(name="av2", bufs=2, space="PSUM"))
    stat_ps = ctx.enter_context(tc.tile_pool(name="stat", bufs=1, space="PSUM"))
    et_pool = ctx.enter_context(tc.tile_pool(name="et", bufs=2 * NPAIR - 2))
    e8_pool = ctx.enter_context(tc.tile_pool(name="e8", bufs=NPAIR + 1))
    rmax_pool = ctx.enter_context(tc.tile_pool(name="rmax", bufs=2))
    fb_pool = ctx.enter_context(tc.tile_pool(name="fb", bufs=2))
    out_pool = ctx.enter_context(tc.tile_pool(name="out_sb", bufs=2))

    def load_mv(start, ntiles):
        """Natural-layout fp8 memory tiles [V*32 | 32] for the AV matmul."""
        mv_t = mv_pool.tile([128, GROUP, VW8], F8, tag="mv")
        for half in range(ntiles // MC):
            base = (start + half * MC) * 128
            nc.sync.dma_start(
                mv_t[:, half * MC : (half + 1) * MC, :],
                d["memv8"][base : base + MC * 128, :].rearrange(
                    "(t p) h -> p t h", p=128
                ),
            )
        return mv_t

    # memT is host-transposed ([H, M] fp16), so the lhsT tiles arrive via
    # plain strided DMAs instead of exclusive-engine XBAR transposes.
    memT_r = d["memT"].rearrange("(c p) m -> p c m", p=128)

    def load_met(start, ntiles):
        """fp16 memT chunks (8 memory tiles each) from the host-transposed array."""
        mets = []
        for c in range(ntiles // MC):
            base = (start + c * MC) * 128
            met8 = met_pool.tile([128, HC, MC * 128], F16, tag="met")
            nc.sync.dma_start(met8[:], memT_r[:, :, base : base + MC * 128])
            mets.append(met8)
        return mets

    # ------------------------------ preamble -------------------------------
    with ExitStack() as pre_ctx:
        pre_w = pre_ctx.enter_context(tc.tile_pool(name="pre_w", bufs=1))
        # wqh/bqh ride the scalar engine's HWDGE queue so both weight loads
        # issue in parallel during framework boot.  Host pre-arranges the
        # weights into the on-chip [p, c, h] layout for contiguous loads.
        wqh = pre_w.tile([128, HC, H], F16)
        wkh = pre_w.tile([128, HC, H], F16)
        # split loads so the first W2 matmul (needs only oc-chunk 0) can
        # start as soon as the first halves land
        wq_r = d["wqh"].rearrange("p (c h) -> p c h", c=HC)
        wk_r = d["wkh"].rearrange("p (c h) -> p c h", c=HC)
        for c in range(HC):
            nc.scalar.dma_start(wqh[:, c : c + 1, :], wq_r[:, c : c + 1, :])
            nc.sync.dma_start(wkh[:, c : c + 1, :], wk_r[:, c : c + 1, :])
        bqh = pre_w.tile([128, HC], F16)
        nc.scalar.dma_start(bqh[:], d["bqh"].rearrange("p c -> p c"))
        featT = pre_w.tile([128, HC, N_LOC], F16)
        nc.sync.dma_start(
            featT[:], d["featT"].rearrange("(c p) n -> p c n", p=128)
        )
        mets0 = []
        for c in range(4):
            # round-0 chunks share the steady-state "met" tag slots (the
            # round-1 prefetches then rotate in as round 0's are consumed)
            met4 = met_pool.tile([128, HC, 4 * 128], F16, tag="met", name=f"met4_{c}")
            nc.sync.dma_start(
                met4[:], memT_r[:, :, c * 4 * 128 : (c + 1) * 4 * 128]
            )
            mets0.append(met4)
        mets = mets0
        mv_t = load_mv(0, GROUP)

        # W2[i, j] = sum_o Wq[o, i] * Wk[o, j]   (fp16 matmul, fp16 result)
        w2r = pre_w.tile([128, HC, H], F16)
        for ic in range(HC):
            ps = s_ps_pool.tile([128, H], F32, tag="sps", name=f"w2ps{ic}")
            for oc in range(HC):
                nc.tensor.matmul(
                    ps[:],
                    wqh[:, oc, ic * 128 : (ic + 1) * 128],
                    wkh[:, oc, :],
                    start=(oc == 0),
                    stop=(oc == HC - 1),
                )
            nc.vector.tensor_copy(w2r[:, ic, :], ps[:])

        # b2T[j] = sum_o Wk[o, j] * bq[o]
        b2full = s_ps_pool.tile([128, H], F32, tag="sps", name="b2ps")
        b2ps = b2full[:, :HC]
        for jc in range(HC):
            for oc in range(HC):
                nc.tensor.matmul(
                    b2ps[:, jc : jc + 1],
                    wkh[:, oc, jc * 128 : (jc + 1) * 128],
                    bqh[:, oc : oc + 1],
                    start=(oc == 0),
                    stop=(oc == HC - 1),
                    skip_group_check=True,
                )
        b2t = pre_w.tile([128, HC], F32)
        nc.vector.tensor_copy(b2t[:], b2ps)

        # q2T[j, n] = sum_i W2[i, j] featT[i, n] + b2T[j]   (fp16 matmul);
        # nh-major so the n-half the first S tiles consume is ready early.
        for nh in range(NH):
            for jc in range(HC):
                ps = s_ps_pool.tile([128, 512], F32, tag="sps", name=f"q2ps{jc}_{nh}")
                for ic in range(HC):
                    nc.tensor.matmul(
                        ps[:],
                        w2r[:, ic, jc * 128 : (jc + 1) * 128],
                        featT[:, ic, nh * 512 : (nh + 1) * 512],
                        start=(ic == 0),
                        stop=(ic == HC - 1),
                    )
                nc.vector.tensor_scalar_add(
                    q2T[:, jc, nh * 512 : (nh + 1) * 512], ps[:], b2t[:, jc : jc + 1]
                )
        pre_ctx.close()  # release wqh/wkh/bqh/w2r/b2t/featT

    # ---------------- main loop over memory-tile rounds --------------------
    # Round r state carried into round r+1's emission window:
    state = {}  # r -> (ets, e8s, mv, rmax)

    def emit_transposes(r):
        """8 PE transposes of rmax_r -> trmax PSUM, feeding the X-reduce."""
        rmax_t = state[r]["rmax"]
        trmax = stat_ps.tile([128, NT, 128], BF16, tag="stat", name=f"trmax{r}")
        for c in range(NT):
            nc.tensor.transpose(
                trmax[:, c, :], rmax_t[:, c * 128 : (c + 1) * 128], ident[:]
            )
        state[r]["trmax"] = trmax

    def emit_stats_dve(r):
        """Per-query scale update for round r (all [128, NT] pp-layout)."""
        nc.vector.tensor_reduce(
            rm[:], state[r]["trmax"][:], mybir.AxisListType.X, AluOpType.max
        )
        # raw = T0 / rowmax; Fnew = min(Fprev, raw); ratio = Fnew * Finv_old
        nc.vector.reciprocal(rm[:], rm[:])
        if r == 0:
            nc.vector.tensor_scalar_mul(Fprev[:], rm[:], T0)
        else:
            nc.vector.tensor_scalar_mul(rm[:], rm[:], T0)
            nc.vector.tensor_tensor(Fprev[:], Fprev[:], rm[:], AluOpType.min)
            nc.vector.tensor_tensor(ratio[:], Fprev[:], Finv[:], AluOpType.mult)
        nc.vector.reciprocal(Finv[:], Fprev[:])
        nc.vector.tensor_copy(f_bf[:], Fprev[:])

    def emit_frow_transposes(r):
        """f_bf [128, NT] -> frow PSUM [1, N_LOC] (row layout for bcast)."""
        frow = stat_ps.tile([1, N_LOC], BF16, tag="stat", name=f"frow{r}")
        for c in range(NT):
            nc.tensor.transpose(
                frow[:, c * 128 : (c + 1) * 128], f_bf[:, c : c + 1], ident[:]
            )
        state[r]["frow"] = frow

    def emit_frow_copy(r):
        frow_sb = fb_pool.tile([1, N_LOC], BF16, tag="frow_sb")
        nc.vector.tensor_copy(frow_sb[:], state[r]["frow"][:])
        state[r]["frow_sb"] = frow_sb

    def emit_bcast(r):
        """Fb2[p, i, n] = f[n] via 1-contraction matmuls + pair-replicated copy."""
        fb2 = fb_pool.tile([128, 2, N_LOC], BF16, tag="fb_sb")
        for nh in range(NH):
            fb_ps = stat_ps.tile([128, 512], F32, tag="stat", name=f"fbps{r}_{nh}")
            nc.tensor.matmul(
                fb_ps[:],
                ones_row[:],
                state[r]["frow_sb"][:, nh * 512 : (nh + 1) * 512],
                start=True,
                stop=True,
            )
            for i in range(2):
                nc.vector.tensor_copy(fb2[:, i, nh * 512 : (nh + 1) * 512], fb_ps[:])
        state[r]["fb2"] = fb2

    def emit_rescale(r):
        """e8 = e4m3(et * F[n]).

        Steady rounds: in-place bf16 multiply on DVE (fast path), then ONE
        gpsimd software-DGE DMA per pair does the bf16 -> fp8 cast (keeping
        the slow fp8-output store path off the vector engine; latency hides
        under the next round's S phase).
        Epilogue round: direct DVE fp8-out multiply per pair -- slower per
        element but lowest latency to first/last pair, which gates AV."""
        fb2 = state[r]["fb2"]
        epilogue = r == N_ROUNDS - 1
        npair = len(state[r]["ets"])
        e8s = [None] * npair
        # steady rounds: all pairs via mult + Pool-cast (latency hidden).
        # epilogue: back half via Pool-cast (emitted first so the casts
        # start early), front half DVE-direct -- both engines in parallel.
        if epilogue:
            order = [(p, True) for p in range(npair // 2, npair)] + [
                (p, False) for p in range(npair // 2)
            ]
        else:
            order = [(p, True) for p in range(npair)]
        for p, pool_path in order:
            et = state[r]["ets"][p]
            e8 = e8_pool.tile([128, 2, N_LOC], F8, tag="e8", name=f"e8_{r}_{p}")
            etf = et[:].rearrange("p a n -> p (a n)")
            fbf = fb2[:].rearrange("p a n -> p (a n)")
            e8f = e8[:].rearrange("p a n -> p (a n)")
            if pool_path:
                nc.vector.tensor_tensor(etf, etf, fbf, AluOpType.mult)
                nc.gpsimd.dma_start(e8f, etf)
            else:
                nc.vector.tensor_tensor(e8f, etf, fbf, AluOpType.mult)
            e8s[p] = e8
        state[r]["e8s"] = e8s

    def emit_av(r):
        """DoubleRow fp8 AV + denominator, flash fixup, merge/store on last."""
        e8s = state[r]["e8s"]
        mv8 = state[r]["mv"]
        npair = len(e8s)
        # consume pairs in availability order (epilogue rescale delivers the
        # back half first); accumulation order is free within a PSUM group
        if r == N_ROUNDS - 1:
            porder = list(range(npair // 2, npair)) + list(range(npair // 2))
        else:
            porder = list(range(npair))
        for nt in range(NT):
            av1 = av1_pool.tile([128, HH + 1], F32, tag="av1")
            av2 = av2_pool.tile([128, HH], F32, tag="av2")
            for k, p in enumerate(porder):
                eb = e8s[p][:, :, nt * 128 : (nt + 1) * 128]
                nc.tensor.matmul(
                    av2[:],
                    eb,
                    mv8[:, 2 * p : 2 * p + 2, 0:HH],
                    start=(k == 0),
                    stop=(k == npair - 1),
                    perf_mode=mybir.MatmulPerfMode.DoubleRow,
                )
                nc.tensor.matmul(
                    av1[:],
                    eb,
                    mv8[:, 2 * p : 2 * p + 2, HH : H + 1],
                    start=(k == 0),
                    stop=(k == npair - 1),
                    perf_mode=mybir.MatmulPerfMode.DoubleRow,
                )
            if r == 0:
                nc.vector.tensor_copy(aug[:, nt, 0:HH], av2[:])
                nc.vector.tensor_copy(aug[:, nt, HH : H + 1], av1[:])
            else:
                nc.vector.tensor_scalar_mul(
                    aug[:, nt, :], aug[:, nt, :], ratio[:, nt : nt + 1]
                )
                nc.vector.tensor_tensor(
                    aug[:, nt, 0:HH], aug[:, nt, 0:HH], av2[:], AluOpType.add
                )
                nc.vector.tensor_tensor(
                    aug[:, nt, HH : H + 1],
                    aug[:, nt, HH : H + 1],
                    av1[:],
                    AluOpType.add,
                )
            if r == N_ROUNDS - 1:
                # denominator complete for this nt: normalize + merge + store
                nc.vector.reciprocal(rh[:, nt : nt + 1], aug[:, nt, H : H + 1])
                nc.vector.tensor_scalar_mul(
                    rh[:, nt : nt + 1], rh[:, nt : nt + 1], 1.0 - MERGE
                )
                # feat already holds MERGE * features (host pre-scaled)
                o = out_pool.tile([128, H], F32, tag="out")
                nc.vector.scalar_tensor_tensor(
                    o[:],
                    aug[:, nt, 0:H],
                    rh[:, nt : nt + 1],
                    feat[:, nt, :],
                    op0=AluOpType.mult,
                    op1=AluOpType.add,
                )
                nc.sync.dma_start(d["out"][nt * 128 : (nt + 1) * 128, :], o[:])

    def emit_s_tiles(g, start, ntiles, tl_range, ets, rmax_t):
        csz = ntiles // len(mets)
        for tl in tl_range:
            met8 = mets[tl // csz]
            t = tl % csz
            pi, i = tl // 2, tl % 2
            if i == 0:
                ets.append(
                    et_pool.tile([128, 2, N_LOC], BF16, tag="et", name=f"et{g}_{pi}")
                )
            et = ets[pi]
            for nh in range(NH):
                sp = s_ps_pool.tile([128, 512], F32, tag="sps")
                for jc in range(HC):
                    nc.tensor.matmul(
                        sp[:],
                        met8[:, jc, t * 128 : (t + 1) * 128],
                        q2T[:, jc, nh * 512 : (nh + 1) * 512],
                        start=(jc == 0),
                        stop=(jc == HC - 1),
                    )
                nc.scalar.activation(
                    et[:, i, nh * 512 : (nh + 1) * 512],
                    sp[:],
                    mybir.ActivationFunctionType.Exp,
                    bias=bias_t[:],
                )
            # running elementwise max for this round's row-max
            if tl == 0:
                nc.vector.tensor_copy(rmax_t[:], et[:, 0, :])
            else:
                nc.vector.tensor_tensor(rmax_t[:], rmax_t[:], et[:, i, :], AluOpType.max)

    for g, (start, ntiles) in enumerate(ROUND_TILES):
        if g + 1 < N_ROUNDS:
            next_mets = load_met(*ROUND_TILES[g + 1])
            next_mv = load_mv(*ROUND_TILES[g + 1])
        if g == 3:
            # merge-side features (fp16, pre-scaled by MERGE) load late
            nc.sync.dma_start(
                feat[:], d["featm"].rearrange("(t p) h -> p t h", p=128)
            )
        r = g - 1  # round whose stats/AV are interleaved with S_g
        ets = []
        rmax_t = rmax_pool.tile([128, N_LOC], BF16, tag="rmax", name=f"rmax{g}")
        state[g] = {"rmax": rmax_t, "mv": mv_t, "ets": ets}

        # The whole stats->rescale chain sits at the TOP of the round: the
        # AV of round r-1 (end of the previous emission) absorbs round r's
        # exp/max tail, so transp8_r starts unstalled, and the rescale's
        # DVE mults run ahead of this round's max ops in the DVE queue --
        # the Pool casts then complete long before AV_r needs the pairs.
        # Round 1 has no preceding AV on the PE to absorb round 0's exp/max
        # tail, so lead with one S tile before the transposes there.
        lead = 1 if r == 0 else 0
        if lead:
            emit_s_tiles(g, start, ntiles, range(0, 1), ets, rmax_t)
        if r >= 0:
            emit_transposes(r)
            emit_stats_dve(r)
        emit_s_tiles(g, start, ntiles, range(lead, lead + 1), ets, rmax_t)
        if r >= 0:
            emit_frow_transposes(r)
            emit_frow_copy(r)
        emit_s_tiles(g, start, ntiles, range(lead + 1, lead + 2), ets, rmax_t)
        if r >= 0:
            emit_bcast(r)
            emit_rescale(r)
        emit_s_tiles(g, start, ntiles, range(lead + 2, ntiles), ets, rmax_t)
        if r >= 0:
            emit_av(r)
            del state[r]
        if g + 1 < N_ROUNDS:
            mets = next_mets
            mv_t = next_mv

    # epilogue: stats + AV for the last round
    r = N_ROUNDS - 1
    emit_transposes(r)
    emit_stats_dve(r)
    emit_frow_transposes(r)
    emit_frow_copy(r)
    emit_bcast(r)
    emit_rescale(r)
    emit_av(r)


def build_module():
    nc = bacc.Bacc("TRN2", target_bir_lowering=False, debug=False)
    d = {
        "featm": nc.dram_tensor("featm", [N_LOC, H], F16, kind="ExternalInput").ap(),
        "featT": nc.dram_tensor("featT", [H, N_LOC], F16, kind="ExternalInput").ap(),
        "memT": nc.dram_tensor("memT", [H, M], F16, kind="ExternalInput").ap(),
        "memv8": nc.dram_tensor("memv8", [M, VW8], F8, kind="ExternalInput").ap(),
        "wqh": nc.dram_tensor("wqh", [128, H // 128 * H], F16, kind="ExternalInput").ap(),
        "wkh": nc.dram_tensor("wkh", [128, H // 128 * H], F16, kind="ExternalInput").ap(),
        "bqh": nc.dram_tensor("bqh", [128, H // 128], F16, kind="ExternalInput").ap(),
        "out": nc.dram_tensor("out", [N_LOC, H], F32, kind="ExternalOutput").ap(),
    }
    with tile.TileContext(nc) as tc, ExitStack() as ctx:
        _emit(nc, tc, ctx, d)
    nc.compile()
    return nc


_CACHED = None


def _warrange(w):  # [512, 512] -> on-chip [p, c*h] layout, contiguous DMA
    w16 = np.asarray(w, dtype=np.float32).astype(np.float16)
    return np.ascontiguousarray(
        w16.reshape(H // 128, 128, H).transpose(1, 0, 2).reshape(128, -1)
    )


def _mem8(mem):  # [M, H] f32 -> [M, VW8] e4m3 of [32*V | 32 | pad]
    out = np.zeros((M, VW8), dtype=ml_dtypes.float8_e4m3)
    scaled = np.clip(mem * VSCALE, -240.0, 240.0)
    out[:, :H] = scaled.astype(ml_dtypes.float8_e4m3)
    out[:, H] = np.float32(VSCALE)
    return out


def kernel(features, memory_features, Wq, bq, Wk, bk=None, **_ignored):
    global _CACHED
    if _CACHED is None:
        _CACHED = build_module()
    nc = _CACHED

    features = np.ascontiguousarray(np.asarray(features, dtype=np.float32))
    memory_features = np.ascontiguousarray(np.asarray(memory_features, dtype=np.float32))
    memT = np.ascontiguousarray(memory_features.T.astype(np.float16))
    memv8 = _mem8(memory_features)
    feath = features.astype(np.float16)  # sharded then transposed per core
    featm = (MERGE * features).astype(np.float16)  # merge-side, pre-scaled
    wqh = _warrange(Wq)
    wkh = _warrange(Wk)
    bqh = np.ascontiguousarray(
        np.asarray(bq, dtype=np.float32).astype(np.float16).reshape(H // 128, 128).T
    )

    in_maps = []
    for c in range(N_CORES):
        in_maps.append(
            {
                "featm": featm[c * N_LOC : (c + 1) * N_LOC],
                "featT": np.ascontiguousarray(
                    feath[c * N_LOC : (c + 1) * N_LOC].T
                ),
                "memT": memT,
                "memv8": memv8,
                "wqh": wqh,
                "wkh": wkh,
                "bqh": bqh,
            }
        )
    res = run_bass_kernel_spmd(nc, in_maps, core_ids=list(range(N_CORES)))
    return np.concatenate([res.results[c]["out"] for c in range(N_CORES)], axis=0)


# revision 36
# speedup vs baseline: 1.0272x; 1.0046x over previous
"""Trainium2 Bass kernel for FeatureAugmentationNetwork2.

Reference computation (N=M=8192, H=512, tau=1, c=0.5):
    q = features @ Wq.T + bq
    k = memory_features @ Wk.T + bk
    attn = softmax(q @ k.T, axis=-1)
    out = c * features + (1-c) * attn @ memory_features

Sharding: features (queries) split across 8 cores on the N axis;
memory_features / weights replicated.  Each core computes its
[1024, 8192] attention slab independently; outputs are concatenated.

Algebraic restructuring (exact):
  - bk adds a per-row constant to the logits -> softmax-invariant -> dropped.
  - S = q @ k.T = (features @ W2 + b2) @ memory.T
    with W2 = Wq.T @ Wk (computed on-chip), b2 = bq @ Wk.
  - softmax without a row max: exp(s - C) with fixed C = 100 into bf16 E
    tiles (logits ~N(0,512); all row maxes land in bf16 range with huge
    margin).
  - The [m, n]-layout exp tile feeds attn.V as lhsT without any transpose;
    the softmax denominator is fused into the AV matmul via a constant
    column appended to V.

fp8 AV path (the big win vs the bf16 version):
  - The AV matmul runs in fp8 (e4m3) with MatmulPerfMode.DoubleRow: each
    matmul instruction contracts over TWO 128-row memory tiles (256 deep)
    at double rate, halving AV's PE time.
  - V is pre-quantized on the host: V8 = e4m3(32*V) with the denominator
    column = 32.0 (the common factor 32 cancels in aug/den).
  - E must be squeezed into e4m3's ~12-nat dynamic window, which requires
    a per-QUERY scale F[n].  Any per-query scale cancels exactly in the
    aug/den ratio, so correctness only needs range-fit, not exactness.
    F[n] = T0 / rowmax_round(E), flash-style across rounds:
    F <- min(F, T0/rowmax_g); the SBUF aug accumulator (including its
    denominator column) is rescaled by F_new/F_old before adding round g.
  - rowmax per round: DVE running elementwise max over the 16 E tiles
    ([m,n] layout), then 8 PE transposes + a DVE X-reduce to get the
    cross-partition max into per-query layout [128, 8].
  - The broadcast of F along the m axis (for the E*F quantize) is a
    1-contraction PE matmul: ones[1,128].T @ f_row[1,512] -> PSUM, copied
    to SBUF bf16; quantize = DVE tensor_tensor mult with fp8 output.

Precision: S path in fp16 as before (logit err ~0.009); E quantization
cancels in aug/den; the V e4m3 mantissa (2.4% rms) dominates the error:
predicted end-to-end rel err 1.84e-2 vs the 2e-2 gate (CPU sim,
calibrated to 0.3% against the bf16 kernel's measured 2.14e-3).

Schedule: stats/rescale/AV for round g-1 are emitted interleaved with
round g's S phase, so the PE never waits on the DVE stats chain: the
PE stream is [S_g | transposes_{g-1} | bcast_{g-1} | AV_{g-1} | S_{g+1}...]
with the DVE work (max tracking, stats, quantize) hidden under S.
"""

from contextlib import ExitStack

import ml_dtypes
import numpy as np

import concourse.bass as bass
import concourse.tile as tile
from concourse import bacc, mybir
from concourse.alu_op_type import AluOpType
from concourse.bass_utils import run_bass_kernel_spmd
from concourse.masks import make_identity

N_CORES = 8
N, M, H = 8192, 8192, 512
N_LOC = N // N_CORES  # 1024 query rows per core
C_OFF = 100.0  # fixed softmax exp offset
MERGE = 0.5
VSCALE = 32.0  # host-side V/denominator scale (cancels in aug/den)
T0 = 120.0  # target scaled row max for fp8 E (<240 with slack)

F32 = mybir.dt.float32
F16 = mybir.dt.float16
BF16 = mybir.dt.bfloat16
F8 = mybir.dt.float8e4

HH = H // 2  # 256
VW8 = H + 8  # fp8 V row: [512 V | 32.0 | 7 pad]

NT = N_LOC // 128  # 8  query-row tiles
MT = M // 128  # 64 memory-row tiles
HC = H // 128  # 4  feature-dim chunks
GROUP = 16  # memory tiles per AV accumulation round
NPAIR = GROUP // 2  # 8  DoubleRow pairs per round
MC = 8  # memory tiles per memT chunk (2 chunks per round)
NH = N_LOC // 512  # 2  n halves (512-wide matmul free dim)
# round partition of the 64 memory tiles (8-tile tail rounds measured SLOWER:
# their S phases are too short to hide the per-round DVE chains)
ROUND_TILES = [(0, 16), (16, 16), (32, 16), (48, 16)]
N_ROUNDS = len(ROUND_TILES)


def _emit(nc, tc, ctx, d):
    main_sb = ctx.enter_context(tc.tile_pool(name="main_sb", bufs=1))
    bias_t = main_sb.tile([128, 1], F32)
    nc.vector.memset(bias_t[:], -C_OFF)
    aug = main_sb.tile([128, NT, H + 1], F32)  # col 512 holds the denominator
    rh = main_sb.tile([128, NT], F32)
    feat = main_sb.tile([128, NT, H], F16)  # holds MERGE * features (host-scaled)
    q2T = main_sb.tile([128, HC, N_LOC], F16)
    # identity for PE transposes (bf16 inputs only)
    ident = main_sb.tile([128, 128], BF16)
    make_identity(nc, ident[:])
    ones_row = main_sb.tile([1, 128], BF16)
    nc.vector.memset(ones_row[:], 1.0)
    # per-query flash scale state, chunk-major [p, nt] (query n = nt*128+p)
    Fprev = main_sb.tile([128, NT], F32)
    Finv = main_sb.tile([128, NT], F32)
    ratio = main_sb.tile([128, NT], F32)
    f_bf = main_sb.tile([128, NT], BF16)
    rm = main_sb.tile([128, NT], F32)

    mv_pool = ctx.enter_context(tc.tile_pool(name="mv", bufs=2))
    met_pool = ctx.enter_context(tc.tile_pool(name="met", bufs=4))
    s_ps_pool = ctx.enter_context(tc.tile_pool(name="sps", bufs=3, space="PSUM"))
    av1_pool = ctx.enter_context(tc.tile_pool(name="av1", bufs=2, space="PSUM"))
    av2_pool = ctx.enter_context(tc.tile_pool(name="av2", bufs=2, space="PSUM"))
    stat_ps = ctx.enter_context(tc.tile_pool(name="stat", bufs=1, space="PSUM"))
    et_pool = ctx.enter_context(tc.tile_pool(name="et", bufs=2 * NPAIR - 2))
    e8_pool = ctx.enter_context(tc.tile_pool(name="e8", bufs=NPAIR + 1))
    rmax_pool = ctx.enter_context(tc.tile_pool(name="rmax", bufs=2))
    fb_pool = ctx.enter_context(tc.tile_pool(name="fb", bufs=2))
    out_pool = ctx.enter_context(tc.tile_pool(name="out_sb", bufs=2))

    def load_mv(start, ntiles):
        """Natural-layout fp8 memory tiles [V*32 | 32] for the AV matmul."""
        mv_t = mv_pool.tile([128, GROUP, VW8], F8, tag="mv")
        for half in range(ntiles // MC):
            base = (start + half * MC) * 128
            nc.sync.dma_start(
                mv_t[:, half * MC : (half + 1) * MC, :],
                d["memv8"][base : base + MC * 128, :].rearrange(
                    "(t p) h -> p t h", p=128
                ),
            )
        return mv_t

    # memT is host-transposed ([H, M] fp16), so the lhsT tiles arrive via
    # plain strided DMAs instead of exclusive-engine XBAR transposes.
    memT_r = d["memT"].rearrange("(c p) m -> p c m", p=128)

    def load_met(start, ntiles):
        """fp16 memT chunks (8 memory tiles each) from the host-transposed array."""
        mets = []
        for c in range(ntiles // MC):
            base = (start + c * MC) * 128
            met8 = met_pool.tile([128, HC, MC * 128], F16, tag="met")
            nc.sync.dma_start(met8[:], memT_r[:, :, base : base + MC * 128])
            mets.append(met8)
        return mets

    # ------------------------------ preamble -------------------------------
    with ExitStack() as pre_ctx:
        pre_w = pre_ctx.enter_context(tc.tile_pool(name="pre_w", bufs=1))
        # wqh/bqh ride the scalar engine's HWDGE queue so both weight loads
        # issue in parallel during framework boot.  Host pre-arranges the
        # weights into the on-chip [p, c, h] layout for contiguous loads.
        wqh = pre_w.tile([128, HC, H], F16)
        wkh = pre_w.tile([128, HC, H], F16)
        # split loads so the first W2 matmul (needs only oc-chunk 0) can
        # start as soon as the first halves land
        wq_r = d["wqh"].rearrange("p (c h) -> p c h", c=HC)
        wk_r = d["wkh"].rearrange("p (c h) -> p c h", c=HC)
        nc.scalar.dma_start(wqh[:, 0:2, :], wq_r[:, 0:2, :])
        nc.sync.dma_start(wkh[:, 0:2, :], wk_r[:, 0:2, :])
        nc.scalar.dma_start(wqh[:, 2:4, :], wq_r[:, 2:4, :])
        nc.sync.dma_start(wkh[:, 2:4, :], wk_r[:, 2:4, :])
        bqh = pre_w.tile([128, HC], F16)
        nc.scalar.dma_start(bqh[:], d["bqh"].rearrange("p c -> p c"))
        featT = pre_w.tile([128, HC, N_LOC], F16)
        nc.sync.dma_start(
            featT[:], d["featT"].rearrange("(c p) n -> p c n", p=128)
        )
        mets0 = []
        for c in range(4):
            # round-0 chunks share the steady-state "met" tag slots (the
            # round-1 prefetches then rotate in as round 0's are consumed)
            met4 = met_pool.tile([128, HC, 4 * 128], F16, tag="met", name=f"met4_{c}")
            nc.sync.dma_start(
                met4[:], memT_r[:, :, c * 4 * 128 : (c + 1) * 4 * 128]
            )
            mets0.append(met4)
        mets = mets0
        mv_t = load_mv(0, GROUP)

        # W2[i, j] = sum_o Wq[o, i] * Wk[o, j]   (fp16 matmul, fp16 result)
        w2r = pre_w.tile([128, HC, H], F16)
        for ic in range(HC):
            ps = s_ps_pool.tile([128, H], F32, tag="sps", name=f"w2ps{ic}")
            for oc in range(HC):
                nc.tensor.matmul(
                    ps[:],
                    wqh[:, oc, ic * 128 : (ic + 1) * 128],
                    wkh[:, oc, :],
                    start=(oc == 0),
                    stop=(oc == HC - 1),
                )
            nc.vector.tensor_copy(w2r[:, ic, :], ps[:])

        # b2T[j] = sum_o Wk[o, j] * bq[o]
        b2full = s_ps_pool.tile([128, H], F32, tag="sps", name="b2ps")
        b2ps = b2full[:, :HC]
        for jc in range(HC):
            for oc in range(HC):
                nc.tensor.matmul(
                    b2ps[:, jc : jc + 1],
                    wkh[:, oc, jc * 128 : (jc + 1) * 128],
                    bqh[:, oc : oc + 1],
                    start=(oc == 0),
                    stop=(oc == HC - 1),
                    skip_group_check=True,
                )
        b2t = pre_w.tile([128, HC], F32)
        nc.vector.tensor_copy(b2t[:], b2ps)

        # q2T[j, n] = sum_i W2[i, j] featT[i, n] + b2T[j]   (fp16 matmul);
        # nh-major so the n-half the first S tiles consume is ready early.
        for nh in range(NH):
            for jc in range(HC):
                ps = s_ps_pool.tile([128, 512], F32, tag="sps", name=f"q2ps{jc}_{nh}")
                for ic in range(HC):
                    nc.tensor.matmul(
                        ps[:],
                        w2r[:, ic, jc * 128 : (jc + 1) * 128],
                        featT[:, ic, nh * 512 : (nh + 1) * 512],
                        start=(ic == 0),
                        stop=(ic == HC - 1),
                    )
                nc.vector.tensor_scalar_add(
                    q2T[:, jc, nh * 512 : (nh + 1) * 512], ps[:], b2t[:, jc : jc + 1]
                )
        pre_ctx.close()  # release wqh/wkh/bqh/w2r/b2t/featT

    # ---------------- main loop over memory-tile rounds --------------------
    # Round r state carried into round r+1's emission window:
    state = {}  # r -> (ets, e8s, mv, rmax)

    def emit_transposes(r):
        """8 PE transposes of rmax_r -> trmax PSUM, feeding the X-reduce."""
        rmax_t = state[r]["rmax"]
        trmax = stat_ps.tile([128, NT, 128], BF16, tag="stat", name=f"trmax{r}")
        for c in range(NT):
            nc.tensor.transpose(
                trmax[:, c, :], rmax_t[:, c * 128 : (c + 1) * 128], ident[:]
            )
        state[r]["trmax"] = trmax

    def emit_stats_dve(r):
        """Per-query scale update for round r (all [128, NT] pp-layout)."""
        nc.vector.tensor_reduce(
            rm[:], state[r]["trmax"][:], mybir.AxisListType.X, AluOpType.max
        )
        # raw = T0 / rowmax; Fnew = min(Fprev, raw); ratio = Fnew * Finv_old
        nc.vector.reciprocal(rm[:], rm[:])
        if r == 0:
            nc.vector.tensor_scalar_mul(Fprev[:], rm[:], T0)
        else:
            nc.vector.tensor_scalar_mul(rm[:], rm[:], T0)
            nc.vector.tensor_tensor(Fprev[:], Fprev[:], rm[:], AluOpType.min)
            nc.vector.tensor_tensor(ratio[:], Fprev[:], Finv[:], AluOpType.mult)
        nc.vector.reciprocal(Finv[:], Fprev[:])
        nc.vector.tensor_copy(f_bf[:], Fprev[:])

    def emit_frow_transposes(r):
        """f_bf [128, NT] -> frow PSUM [1, N_LOC] (row layout for bcast)."""
        frow = stat_ps.tile([1, N_LOC], BF16, tag="stat", name=f"frow{r}")
        for c in range(NT):
            nc.tensor.transpose(
                frow[:, c * 128 : (c + 1) * 128], f_bf[:, c : c + 1], ident[:]
            )
        state[r]["frow"] = frow

    def emit_frow_copy(r):
        frow_sb = fb_pool.tile([1, N_LOC], BF16, tag="frow_sb")
        nc.vector.tensor_copy(frow_sb[:], state[r]["frow"][:])
        state[r]["frow_sb"] = frow_sb

    def emit_bcast(r):
        """Fb2[p, i, n] = f[n] via 1-contraction matmuls + pair-replicated copy."""
        fb2 = fb_pool.tile([128, 2, N_LOC], BF16, tag="fb_sb")
        for nh in range(NH):
            fb_ps = stat_ps.tile([128, 512], F32, tag="stat", name=f"fbps{r}_{nh}")
            nc.tensor.matmul(
                fb_ps[:],
                ones_row[:],
                state[r]["frow_sb"][:, nh * 512 : (nh + 1) * 512],
                start=True,
                stop=True,
            )
            for i in range(2):
                nc.vector.tensor_copy(fb2[:, i, nh * 512 : (nh + 1) * 512], fb_ps[:])
        state[r]["fb2"] = fb2

    def emit_rescale(r):
        """e8 = e4m3(et * F[n]).

        Steady rounds: in-place bf16 multiply on DVE (fast path), then ONE
        gpsimd software-DGE DMA per pair does the bf16 -> fp8 cast (keeping
        the slow fp8-output store path off the vector engine; latency hides
        under the next round's S phase).
        Epilogue round: direct DVE fp8-out multiply per pair -- slower per
        element but lowest latency to first/last pair, which gates AV."""
        fb2 = state[r]["fb2"]
        epilogue = r == N_ROUNDS - 1
        npair = len(state[r]["ets"])
        e8s = [None] * npair
        # steady rounds: all pairs via mult + Pool-cast (latency hidden).
        # epilogue: back half via Pool-cast (emitted first so the casts
        # start early), front half DVE-direct -- both engines in parallel.
        if epilogue:
            order = [(p, True) for p in range(npair // 2, npair)] + [
                (p, False) for p in range(npair // 2)
            ]
        else:
            order = [(p, True) for p in range(npair)]
        if epilogue:
            # nh-split: deliver every pair's nh0 half first -- AV's first
            # query-tiles depend only on nh0 (subtile deps), so AV starts
            # ~5us earlier while the nh1 halves land behind it
            e8l = {}
            for p, pool_path in order:
                e8l[p] = e8_pool.tile(
                    [128, 2, N_LOC], F8, tag="e8", name=f"e8_{r}_{p}"
                )
                e8s[p] = e8l[p]
            for nh in range(NH):
                sl = slice(nh * 512, nh * 512 + 512)
                for p, pool_path in order:
                    et = state[r]["ets"][p]
                    if pool_path:
                        nc.vector.tensor_tensor(
                            et[:, :, sl], et[:, :, sl], fb2[:, :, sl], AluOpType.mult
                        )
                        nc.gpsimd.dma_start(e8l[p][:, :, sl], et[:, :, sl])
                    else:
                        nc.vector.tensor_tensor(
                            e8l[p][:, :, sl], et[:, :, sl], fb2[:, :, sl],
                            AluOpType.mult,
                        )
        else:
            for p, pool_path in order:
                et = state[r]["ets"][p]
                e8 = e8_pool.tile([128, 2, N_LOC], F8, tag="e8", name=f"e8_{r}_{p}")
                etf = et[:].rearrange("p a n -> p (a n)")
                fbf = fb2[:].rearrange("p a n -> p (a n)")
                e8f = e8[:].rearrange("p a n -> p (a n)")
                if pool_path:
                    nc.vector.tensor_tensor(etf, etf, fbf, AluOpType.mult)
                    nc.gpsimd.dma_start(e8f, etf)
                else:
                    nc.vector.tensor_tensor(e8f, etf, fbf, AluOpType.mult)
                e8s[p] = e8
        state[r]["e8s"] = e8s

    def emit_av(r):
        """DoubleRow fp8 AV + denominator, flash fixup, merge/store on last."""
        e8s = state[r]["e8s"]
        mv8 = state[r]["mv"]
        npair = len(e8s)
        # consume pairs in availability order (epilogue rescale delivers the
        # back half first); accumulation order is free within a PSUM group
        if r == N_ROUNDS - 1:
            porder = list(range(npair // 2, npair)) + list(range(npair // 2))
        else:
            porder = list(range(npair))
        for nt in range(NT):
            av1 = av1_pool.tile([128, HH + 1], F32, tag="av1")
            av2 = av2_pool.tile([128, HH], F32, tag="av2")
            for k, p in enumerate(porder):
                eb = e8s[p][:, :, nt * 128 : (nt + 1) * 128]
                nc.tensor.matmul(
                    av2[:],
                    eb,
                    mv8[:, 2 * p : 2 * p + 2, 0:HH],
                    start=(k == 0),
                    stop=(k == npair - 1),
                    perf_mode=mybir.MatmulPerfMode.DoubleRow,
                )
                nc.tensor.matmul(
                    av1[:],
                    eb,
                    mv8[:, 2 * p : 2 * p + 2, HH : H + 1],
                    start=(k == 0),
                    stop=(k == npair - 1),
                    perf_mode=mybir.MatmulPerfMode.DoubleRow,
                )
            if r == 0:
                nc.vector.tensor_copy(aug[:, nt, 0:HH], av2[:])
                nc.vector.tensor_copy(aug[:, nt, HH : H + 1], av1[:])
            else:
                nc.vector.tensor_scalar_mul(
                    aug[:, nt, :], aug[:, nt, :], ratio[:, nt : nt + 1]
                )
                nc.vector.tensor_tensor(
                    aug[:, nt, 0:HH], aug[:, nt, 0:HH], av2[:], AluOpType.add
                )
                nc.vector.tensor_tensor(
                    aug[:, nt, HH : H + 1],
                    aug[:, nt, HH : H + 1],
                    av1[:],
                    AluOpType.add,
                )
            if r == N_ROUNDS - 1:
                # denominator complete for this nt: normalize + merge + store
                nc.vector.reciprocal(rh[:, nt : nt + 1], aug[:, nt, H : H + 1])
                nc.vector.tensor_scalar_mul(
                    rh[:, nt : nt + 1], rh[:, nt : nt + 1], 1.0 - MERGE
                )
                # feat already holds MERGE * features (host pre-scaled)
                o = out_pool.tile([128, H], F32, tag="out")
                nc.vector.scalar_tensor_tensor(
                    o[:],
                    aug[:, nt, 0:H],
                    rh[:, nt : nt + 1],
                    feat[:, nt, :],
                    op0=AluOpType.mult,
                    op1=AluOpType.add,
                )
                nc.sync.dma_start(d["out"][nt * 128 : (nt + 1) * 128, :], o[:])

    def emit_s_tiles(g, start, ntiles, tl_range, ets, rmax_t):
        csz = ntiles // len(mets)
        for tl in tl_range:
            met8 = mets[tl // csz]
            t = tl % csz
            pi, i = tl // 2, tl % 2
            if i == 0:
                ets.append(
                    et_pool.tile([128, 2, N_LOC], BF16, tag="et", name=f"et{g}_{pi}")
                )
            et = ets[pi]
            for nh in range(NH):
                sp = s_ps_pool.tile([128, 512], F32, tag="sps")
                for jc in range(HC):
                    nc.tensor.matmul(
                        sp[:],
                        met8[:, jc, t * 128 : (t + 1) * 128],
                        q2T[:, jc, nh * 512 : (nh + 1) * 512],
                        start=(jc == 0),
                        stop=(jc == HC - 1),
                    )
                nc.scalar.activation(
                    et[:, i, nh * 512 : (nh + 1) * 512],
                    sp[:],
                    mybir.ActivationFunctionType.Exp,
                    bias=bias_t[:],
                )
            # running elementwise max for this round's row-max
            if tl == 0:
                nc.vector.tensor_copy(rmax_t[:], et[:, 0, :])
            else:
                nc.vector.tensor_tensor(rmax_t[:], rmax_t[:], et[:, i, :], AluOpType.max)

    for g, (start, ntiles) in enumerate(ROUND_TILES):
        if g + 1 < N_ROUNDS:
            next_mets = load_met(*ROUND_TILES[g + 1])
            next_mv = load_mv(*ROUND_TILES[g + 1])
        if g == 3:
            # merge-side features (fp16, pre-scaled by MERGE) load late
            nc.sync.dma_start(
                feat[:], d["featm"].rearrange("(t p) h -> p t h", p=128)
            )
        r = g - 1  # round whose stats/AV are interleaved with S_g
        ets = []
        rmax_t = rmax_pool.tile([128, N_LOC], BF16, tag="rmax", name=f"rmax{g}")
        state[g] = {"rmax": rmax_t, "mv": mv_t, "ets": ets}

        # The whole stats->rescale chain sits at the TOP of the round: the
        # AV of round r-1 (end of the previous emission) absorbs round r's
        # exp/max tail, so transp8_r starts unstalled, and the rescale's
        # DVE mults run ahead of this round's max ops in the DVE queue --
        # the Pool casts then complete long before AV_r needs the pairs.
        # Round 1 has no preceding AV on the PE to absorb round 0's exp/max
        # tail, so lead with one S tile before the transposes there.
        lead = 1 if r == 0 else 0
        if lead:
            emit_s_tiles(g, start, ntiles, range(0, 1), ets, rmax_t)
        if r >= 0:
            emit_transposes(r)
            emit_stats_dve(r)
        emit_s_tiles(g, start, ntiles, range(lead, lead + 1), ets, rmax_t)
        if r >= 0:
            emit_frow_transposes(r)
            emit_frow_copy(r)
        emit_s_tiles(g, start, ntiles, range(lead + 1, lead + 2), ets, rmax_t)
        if r >= 0:
            emit_bcast(r)
            emit_rescale(r)
        emit_s_tiles(g, start, ntiles, range(lead + 2, ntiles), ets, rmax_t)
        if r >= 0:
            emit_av(r)
            del state[r]
        if g + 1 < N_ROUNDS:
            mets = next_mets
            mv_t = next_mv

    # epilogue: stats + AV for the last round
    r = N_ROUNDS - 1
    emit_transposes(r)
    emit_stats_dve(r)
    emit_frow_transposes(r)
    emit_frow_copy(r)
    emit_bcast(r)
    emit_rescale(r)
    emit_av(r)


def build_module():
    nc = bacc.Bacc("TRN2", target_bir_lowering=False, debug=False)
    d = {
        "featm": nc.dram_tensor("featm", [N_LOC, H], F16, kind="ExternalInput").ap(),
        "featT": nc.dram_tensor("featT", [H, N_LOC], F16, kind="ExternalInput").ap(),
        "memT": nc.dram_tensor("memT", [H, M], F16, kind="ExternalInput").ap(),
        "memv8": nc.dram_tensor("memv8", [M, VW8], F8, kind="ExternalInput").ap(),
        "wqh": nc.dram_tensor("wqh", [128, H // 128 * H], F16, kind="ExternalInput").ap(),
        "wkh": nc.dram_tensor("wkh", [128, H // 128 * H], F16, kind="ExternalInput").ap(),
        "bqh": nc.dram_tensor("bqh", [128, H // 128], F16, kind="ExternalInput").ap(),
        "out": nc.dram_tensor("out", [N_LOC, H], F32, kind="ExternalOutput").ap(),
    }
    with tile.TileContext(nc) as tc, ExitStack() as ctx:
        _emit(nc, tc, ctx, d)
    nc.compile()
    return nc


_CACHED = None


def _warrange(w):  # [512, 512] -> on-chip [p, c*h] layout, contiguous DMA
    w16 = np.asarray(w, dtype=np.float32).astype(np.float16)
    return np.ascontiguousarray(
        w16.reshape(H // 128, 128, H).transpose(1, 0, 2).reshape(128, -1)
    )


def _mem8(mem):  # [M, H] f32 -> [M, VW8] e4m3 of [32*V | 32 | pad]
    out = np.zeros((M, VW8), dtype=ml_dtypes.float8_e4m3)
    scaled = np.clip(mem * VSCALE, -240.0, 240.0)
    out[:, :H] = scaled.astype(ml_dtypes.float8_e4m3)
    out[:, H] = np.float32(VSCALE)
    return out


def kernel(features, memory_features, Wq, bq, Wk, bk=None, **_ignored):
    global _CACHED
    if _CACHED is None:
        _CACHED = build_module()
    nc = _CACHED

    features = np.ascontiguousarray(np.asarray(features, dtype=np.float32))
    memory_features = np.ascontiguousarray(np.asarray(memory_features, dtype=np.float32))
    memT = np.ascontiguousarray(memory_features.T.astype(np.float16))
    memv8 = _mem8(memory_features)
    feath = features.astype(np.float16)  # sharded then transposed per core
    featm = (MERGE * features).astype(np.float16)  # merge-side, pre-scaled
    wqh = _warrange(Wq)
    wkh = _warrange(Wk)
    bqh = np.ascontiguousarray(
        np.asarray(bq, dtype=np.float32).astype(np.float16).reshape(H // 128, 128).T
    )

    in_maps = []
    for c in range(N_CORES):
        in_maps.append(
            {
                "featm": featm[c * N_LOC : (c + 1) * N_LOC],
                "featT": np.ascontiguousarray(
                    feath[c * N_LOC : (c + 1) * N_LOC].T
                ),
                "memT": memT,
                "memv8": memv8,
                "wqh": wqh,
                "wkh": wkh,
                "bqh": bqh,
            }
        )
    res = run_bass_kernel_spmd(nc, in_maps, core_ids=list(range(N_CORES)))
    return np.concatenate([res.results[c]["out"] for c in range(N_CORES)], axis=0)


# revision 37
# speedup vs baseline: 1.0362x; 1.0088x over previous
"""Trainium2 Bass kernel for FeatureAugmentationNetwork2.

Reference computation (N=M=8192, H=512, tau=1, c=0.5):
    q = features @ Wq.T + bq
    k = memory_features @ Wk.T + bk
    attn = softmax(q @ k.T, axis=-1)
    out = c * features + (1-c) * attn @ memory_features

Sharding: features (queries) split across 8 cores on the N axis;
memory_features / weights replicated.  Each core computes its
[1024, 8192] attention slab independently; outputs are concatenated.

Algebraic restructuring (exact):
  - bk adds a per-row constant to the logits -> softmax-invariant -> dropped.
  - S = q @ k.T = (features @ W2 + b2) @ memory.T
    with W2 = Wq.T @ Wk (computed on-chip), b2 = bq @ Wk.
  - softmax without a row max: exp(s - C) with fixed C = 100 into bf16 E
    tiles (logits ~N(0,512); all row maxes land in bf16 range with huge
    margin).
  - The [m, n]-layout exp tile feeds attn.V as lhsT without any transpose;
    the softmax denominator is fused into the AV matmul via a constant
    column appended to V.

fp8 AV path (the big win vs the bf16 version):
  - The AV matmul runs in fp8 (e4m3) with MatmulPerfMode.DoubleRow: each
    matmul instruction contracts over TWO 128-row memory tiles (256 deep)
    at double rate, halving AV's PE time.
  - V is pre-quantized on the host: V8 = e4m3(32*V) with the denominator
    column = 32.0 (the common factor 32 cancels in aug/den).
  - E must be squeezed into e4m3's ~12-nat dynamic window, which requires
    a per-QUERY scale F[n].  Any per-query scale cancels exactly in the
    aug/den ratio, so correctness only needs range-fit, not exactness.
    F[n] = T0 / rowmax_round(E), flash-style across rounds:
    F <- min(F, T0/rowmax_g); the SBUF aug accumulator (including its
    denominator column) is rescaled by F_new/F_old before adding round g.
  - rowmax per round: DVE running elementwise max over the 16 E tiles
    ([m,n] layout), then 8 PE transposes + a DVE X-reduce to get the
    cross-partition max into per-query layout [128, 8].
  - The broadcast of F along the m axis (for the E*F quantize) is a
    1-contraction PE matmul: ones[1,128].T @ f_row[1,512] -> PSUM, copied
    to SBUF bf16; quantize = DVE tensor_tensor mult with fp8 output.

Precision: S path in fp16 as before (logit err ~0.009); E quantization
cancels in aug/den; the V e4m3 mantissa (2.4% rms) dominates the error:
predicted end-to-end rel err 1.84e-2 vs the 2e-2 gate (CPU sim,
calibrated to 0.3% against the bf16 kernel's measured 2.14e-3).

Schedule: stats/rescale/AV for round g-1 are emitted interleaved with
round g's S phase, so the PE never waits on the DVE stats chain: the
PE stream is [S_g | transposes_{g-1} | bcast_{g-1} | AV_{g-1} | S_{g+1}...]
with the DVE work (max tracking, stats, quantize) hidden under S.
"""

from contextlib import ExitStack

import ml_dtypes
import numpy as np

import concourse.bass as bass
import concourse.tile as tile
from concourse import bacc, mybir
from concourse.alu_op_type import AluOpType
from concourse.bass_utils import run_bass_kernel_spmd
from concourse.masks import make_identity

N_CORES = 8
N, M, H = 8192, 8192, 512
N_LOC = N // N_CORES  # 1024 query rows per core
C_OFF = 100.0  # fixed softmax exp offset
MERGE = 0.5
VSCALE = 32.0  # host-side V/denominator scale (cancels in aug/den)
T0 = 120.0  # target scaled row max for fp8 E (<240 with slack)

F32 = mybir.dt.float32
F16 = mybir.dt.float16
BF16 = mybir.dt.bfloat16
F8 = mybir.dt.float8e4

HH = H // 2  # 256
VW8 = H + 8  # fp8 V row: [512 V | 32.0 | 7 pad]

NT = N_LOC // 128  # 8  query-row tiles
MT = M // 128  # 64 memory-row tiles
HC = H // 128  # 4  feature-dim chunks
GROUP = 16  # memory tiles per AV accumulation round
NPAIR = GROUP // 2  # 8  DoubleRow pairs per round
MC = 8  # memory tiles per memT chunk (2 chunks per round)
NH = N_LOC // 512  # 2  n halves (512-wide matmul free dim)
# round partition of the 64 memory tiles (8-tile tail rounds measured SLOWER:
# their S phases are too short to hide the per-round DVE chains)
ROUND_TILES = [(0, 16), (16, 16), (32, 16), (48, 16)]
N_ROUNDS = len(ROUND_TILES)


def _emit(nc, tc, ctx, d):
    main_sb = ctx.enter_context(tc.tile_pool(name="main_sb", bufs=1))
    bias_t = main_sb.tile([128, 1], F32)
    nc.vector.memset(bias_t[:], -C_OFF)
    aug = main_sb.tile([128, NT, H + 1], F32)  # col 512 holds the denominator
    rh = main_sb.tile([128, NT], F32)
    feat = main_sb.tile([128, NT, H], F16)  # holds MERGE * features (host-scaled)
    q2T = main_sb.tile([128, HC, N_LOC], F16)
    # identity for PE transposes (bf16 inputs only)
    ident = main_sb.tile([128, 128], BF16)
    make_identity(nc, ident[:])
    ones_row = main_sb.tile([1, 128], BF16)
    nc.vector.memset(ones_row[:], 1.0)
    # per-query flash scale state, chunk-major [p, nt] (query n = nt*128+p)
    Fprev = main_sb.tile([128, NT], F32)
    Finv = main_sb.tile([128, NT], F32)
    ratio = main_sb.tile([128, NT], F32)
    f_bf = main_sb.tile([128, NT], BF16)
    rm = main_sb.tile([128, NT], F32)

    mv_pool = ctx.enter_context(tc.tile_pool(name="mv", bufs=2))
    met_pool = ctx.enter_context(tc.tile_pool(name="met", bufs=4))
    s_ps_pool = ctx.enter_context(tc.tile_pool(name="sps", bufs=3, space="PSUM"))
    av1_pool = ctx.enter_context(tc.tile_pool(name="av1", bufs=2, space="PSUM"))
    av2_pool = ctx.enter_context(tc.tile_pool(name="av2", bufs=2, space="PSUM"))
    stat_ps = ctx.enter_context(tc.tile_pool(name="stat", bufs=1, space="PSUM"))
    et_pool = ctx.enter_context(tc.tile_pool(name="et", bufs=2 * NPAIR - 2))
    e8_pool = ctx.enter_context(tc.tile_pool(name="e8", bufs=NPAIR + 1))
    rmax_pool = ctx.enter_context(tc.tile_pool(name="rmax", bufs=2))
    fb_pool = ctx.enter_context(tc.tile_pool(name="fb", bufs=2))
    out_pool = ctx.enter_context(tc.tile_pool(name="out_sb", bufs=2))

    def load_mv(start, ntiles):
        """Natural-layout fp8 memory tiles [V*32 | 32] for the AV matmul."""
        mv_t = mv_pool.tile([128, GROUP, VW8], F8, tag="mv")
        for half in range(ntiles // MC):
            base = (start + half * MC) * 128
            nc.sync.dma_start(
                mv_t[:, half * MC : (half + 1) * MC, :],
                d["memv8"][base : base + MC * 128, :].rearrange(
                    "(t p) h -> p t h", p=128
                ),
            )
        return mv_t

    # memT is host-transposed ([H, M] fp16), so the lhsT tiles arrive via
    # plain strided DMAs instead of exclusive-engine XBAR transposes.
    memT_r = d["memT"].rearrange("(c p) m -> p c m", p=128)

    def load_met(start, ntiles):
        """fp16 memT chunks (8 memory tiles each) from the host-transposed array."""
        mets = []
        for c in range(ntiles // MC):
            base = (start + c * MC) * 128
            met8 = met_pool.tile([128, HC, MC * 128], F16, tag="met")
            nc.sync.dma_start(met8[:], memT_r[:, :, base : base + MC * 128])
            mets.append(met8)
        return mets

    # ------------------------------ preamble -------------------------------
    with ExitStack() as pre_ctx:
        pre_w = pre_ctx.enter_context(tc.tile_pool(name="pre_w", bufs=1))
        # wqh/bqh ride the scalar engine's HWDGE queue so both weight loads
        # issue in parallel during framework boot.  Host pre-arranges the
        # weights into the on-chip [p, c, h] layout for contiguous loads.
        wqh = pre_w.tile([128, HC, H], F16)
        wkh = pre_w.tile([128, HC, H], F16)
        # split loads so the first W2 matmul (needs only oc-chunk 0) can
        # start as soon as the first halves land
        wq_r = d["wqh"].rearrange("p (c h) -> p c h", c=HC)
        wk_r = d["wkh"].rearrange("p (c h) -> p c h", c=HC)
        nc.scalar.dma_start(wqh[:, 0:2, :], wq_r[:, 0:2, :])
        nc.sync.dma_start(wkh[:, 0:2, :], wk_r[:, 0:2, :])
        nc.scalar.dma_start(wqh[:, 2:4, :], wq_r[:, 2:4, :])
        nc.sync.dma_start(wkh[:, 2:4, :], wk_r[:, 2:4, :])
        bqh = pre_w.tile([128, HC], F16)
        nc.scalar.dma_start(bqh[:], d["bqh"].rearrange("p c -> p c"))
        featT = pre_w.tile([128, HC, N_LOC], F16)
        nc.sync.dma_start(
            featT[:], d["featT"].rearrange("(c p) n -> p c n", p=128)
        )
        mets0 = []
        for c in range(4):
            # round-0 chunks share the steady-state "met" tag slots (the
            # round-1 prefetches then rotate in as round 0's are consumed)
            met4 = met_pool.tile([128, HC, 4 * 128], F16, tag="met", name=f"met4_{c}")
            nc.sync.dma_start(
                met4[:], memT_r[:, :, c * 4 * 128 : (c + 1) * 4 * 128]
            )
            mets0.append(met4)
        mets = mets0
        mv_t = load_mv(0, GROUP)

        # W2[i, j] = sum_o Wq[o, i] * Wk[o, j]   (fp16 matmul, fp16 result)
        w2r = pre_w.tile([128, HC, H], F16)
        for ic in range(HC):
            ps = s_ps_pool.tile([128, H], F32, tag="sps", name=f"w2ps{ic}")
            for oc in range(HC):
                nc.tensor.matmul(
                    ps[:],
                    wqh[:, oc, ic * 128 : (ic + 1) * 128],
                    wkh[:, oc, :],
                    start=(oc == 0),
                    stop=(oc == HC - 1),
                )
            nc.vector.tensor_copy(w2r[:, ic, :], ps[:])

        # b2T[j] = sum_o Wk[o, j] * bq[o]
        b2full = s_ps_pool.tile([128, H], F32, tag="sps", name="b2ps")
        b2ps = b2full[:, :HC]
        for jc in range(HC):
            for oc in range(HC):
                nc.tensor.matmul(
                    b2ps[:, jc : jc + 1],
                    wkh[:, oc, jc * 128 : (jc + 1) * 128],
                    bqh[:, oc : oc + 1],
                    start=(oc == 0),
                    stop=(oc == HC - 1),
                    skip_group_check=True,
                )
        b2t = pre_w.tile([128, HC], F32)
        nc.vector.tensor_copy(b2t[:], b2ps)

        # q2T[j, n] = sum_i W2[i, j] featT[i, n] + b2T[j]   (fp16 matmul);
        # nh-major so the n-half the first S tiles consume is ready early.
        for nh in range(NH):
            for jc in range(HC):
                ps = s_ps_pool.tile([128, 512], F32, tag="sps", name=f"q2ps{jc}_{nh}")
                for ic in range(HC):
                    nc.tensor.matmul(
                        ps[:],
                        w2r[:, ic, jc * 128 : (jc + 1) * 128],
                        featT[:, ic, nh * 512 : (nh + 1) * 512],
                        start=(ic == 0),
                        stop=(ic == HC - 1),
                    )
                nc.vector.tensor_scalar_add(
                    q2T[:, jc, nh * 512 : (nh + 1) * 512], ps[:], b2t[:, jc : jc + 1]
                )
        pre_ctx.close()  # release wqh/wkh/bqh/w2r/b2t/featT

    # ---------------- main loop over memory-tile rounds --------------------
    # Round r state carried into round r+1's emission window:
    state = {}  # r -> (ets, e8s, mv, rmax)

    def emit_transposes(r):
        """8 PE transposes of rmax_r -> trmax PSUM, feeding the X-reduce."""
        rmax_t = state[r]["rmax"]
        trmax = stat_ps.tile([128, NT, 128], BF16, tag="stat", name=f"trmax{r}")
        for c in range(NT):
            nc.tensor.transpose(
                trmax[:, c, :], rmax_t[:, c * 128 : (c + 1) * 128], ident[:]
            )
        state[r]["trmax"] = trmax

    def emit_stats_dve(r):
        """Per-query scale update for round r (all [128, NT] pp-layout)."""
        nc.vector.tensor_reduce(
            rm[:], state[r]["trmax"][:], mybir.AxisListType.X, AluOpType.max
        )
        # raw = T0 / rowmax; Fnew = min(Fprev, raw); ratio = Fnew * Finv_old
        nc.vector.reciprocal(rm[:], rm[:])
        if r == 0:
            nc.vector.tensor_scalar_mul(Fprev[:], rm[:], T0)
        else:
            nc.vector.tensor_scalar_mul(rm[:], rm[:], T0)
            nc.vector.tensor_tensor(Fprev[:], Fprev[:], rm[:], AluOpType.min)
            nc.vector.tensor_tensor(ratio[:], Fprev[:], Finv[:], AluOpType.mult)
        nc.vector.reciprocal(Finv[:], Fprev[:])
        nc.vector.tensor_copy(f_bf[:], Fprev[:])

    def emit_frow_transposes(r):
        """f_bf [128, NT] -> frow PSUM [1, N_LOC] (row layout for bcast)."""
        frow = stat_ps.tile([1, N_LOC], BF16, tag="stat", name=f"frow{r}")
        for c in range(NT):
            nc.tensor.transpose(
                frow[:, c * 128 : (c + 1) * 128], f_bf[:, c : c + 1], ident[:]
            )
        state[r]["frow"] = frow

    def emit_frow_copy(r):
        frow_sb = fb_pool.tile([1, N_LOC], BF16, tag="frow_sb")
        nc.vector.tensor_copy(frow_sb[:], state[r]["frow"][:])
        state[r]["frow_sb"] = frow_sb

    def emit_bcast(r):
        """Fb2[p, i, n] = f[n] via 1-contraction matmuls + pair-replicated copy."""
        fb2 = fb_pool.tile([128, 2, N_LOC], BF16, tag="fb_sb")
        for nh in range(NH):
            fb_ps = stat_ps.tile([128, 512], F32, tag="stat", name=f"fbps{r}_{nh}")
            nc.tensor.matmul(
                fb_ps[:],
                ones_row[:],
                state[r]["frow_sb"][:, nh * 512 : (nh + 1) * 512],
                start=True,
                stop=True,
            )
            for i in range(2):
                nc.vector.tensor_copy(fb2[:, i, nh * 512 : (nh + 1) * 512], fb_ps[:])
        state[r]["fb2"] = fb2

    def emit_rescale(r):
        """e8 = e4m3(et * F[n]).

        Steady rounds: in-place bf16 multiply on DVE (fast path), then ONE
        gpsimd software-DGE DMA per pair does the bf16 -> fp8 cast (keeping
        the slow fp8-output store path off the vector engine; latency hides
        under the next round's S phase).
        Epilogue round: direct DVE fp8-out multiply per pair -- slower per
        element but lowest latency to first/last pair, which gates AV."""
        fb2 = state[r]["fb2"]
        epilogue = r == N_ROUNDS - 1
        npair = len(state[r]["ets"])
        e8s = [None] * npair
        # steady rounds: all pairs via mult + Pool-cast (latency hidden).
        # epilogue: back half via Pool-cast (emitted first so the casts
        # start early), front half DVE-direct -- both engines in parallel.
        if epilogue:
            # all pairs via fast bf16 mult + Pool cast: the DVE queue stays
            # clear of slow fp8-out ops so the AV-phase aug adds never wait
            order = [(p, True) for p in range(npair)]
        else:
            order = [(p, True) for p in range(npair)]
        if epilogue:
            # nh-split: deliver every pair's nh0 half first -- AV's first
            # query-tiles depend only on nh0 (subtile deps), so AV starts
            # ~5us earlier while the nh1 halves land behind it
            e8l = {}
            for p, pool_path in order:
                e8l[p] = e8_pool.tile(
                    [128, 2, N_LOC], F8, tag="e8", name=f"e8_{r}_{p}"
                )
                e8s[p] = e8l[p]
            for nh in range(NH):
                sl = slice(nh * 512, nh * 512 + 512)
                for p, pool_path in order:
                    et = state[r]["ets"][p]
                    if pool_path:
                        nc.vector.tensor_tensor(
                            et[:, :, sl], et[:, :, sl], fb2[:, :, sl], AluOpType.mult
                        )
                        nc.gpsimd.dma_start(e8l[p][:, :, sl], et[:, :, sl])
                    else:
                        nc.vector.tensor_tensor(
                            e8l[p][:, :, sl], et[:, :, sl], fb2[:, :, sl],
                            AluOpType.mult,
                        )
        else:
            for p, pool_path in order:
                et = state[r]["ets"][p]
                e8 = e8_pool.tile([128, 2, N_LOC], F8, tag="e8", name=f"e8_{r}_{p}")
                etf = et[:].rearrange("p a n -> p (a n)")
                fbf = fb2[:].rearrange("p a n -> p (a n)")
                e8f = e8[:].rearrange("p a n -> p (a n)")
                if pool_path:
                    nc.vector.tensor_tensor(etf, etf, fbf, AluOpType.mult)
                    nc.gpsimd.dma_start(e8f, etf)
                else:
                    nc.vector.tensor_tensor(e8f, etf, fbf, AluOpType.mult)
                e8s[p] = e8
        state[r]["e8s"] = e8s

    def emit_av(r):
        """DoubleRow fp8 AV + denominator, flash fixup, merge/store on last."""
        e8s = state[r]["e8s"]
        mv8 = state[r]["mv"]
        npair = len(e8s)
        # consume pairs in availability order (epilogue rescale delivers the
        # back half first); accumulation order is free within a PSUM group
        if r == N_ROUNDS - 1:
            porder = list(range(npair // 2, npair)) + list(range(npair // 2))
        else:
            porder = list(range(npair))
        for nt in range(NT):
            av1 = av1_pool.tile([128, HH + 1], F32, tag="av1")
            av2 = av2_pool.tile([128, HH], F32, tag="av2")
            for k, p in enumerate(porder):
                eb = e8s[p][:, :, nt * 128 : (nt + 1) * 128]
                nc.tensor.matmul(
                    av2[:],
                    eb,
                    mv8[:, 2 * p : 2 * p + 2, 0:HH],
                    start=(k == 0),
                    stop=(k == npair - 1),
                    perf_mode=mybir.MatmulPerfMode.DoubleRow,
                )
                nc.tensor.matmul(
                    av1[:],
                    eb,
                    mv8[:, 2 * p : 2 * p + 2, HH : H + 1],
                    start=(k == 0),
                    stop=(k == npair - 1),
                    perf_mode=mybir.MatmulPerfMode.DoubleRow,
                )
            if r == 0:
                nc.vector.tensor_copy(aug[:, nt, 0:HH], av2[:])
                nc.vector.tensor_copy(aug[:, nt, HH : H + 1], av1[:])
            else:
                nc.vector.tensor_scalar_mul(
                    aug[:, nt, :], aug[:, nt, :], ratio[:, nt : nt + 1]
                )
                nc.vector.tensor_tensor(
                    aug[:, nt, 0:HH], aug[:, nt, 0:HH], av2[:], AluOpType.add
                )
                nc.vector.tensor_tensor(
                    aug[:, nt, HH : H + 1],
                    aug[:, nt, HH : H + 1],
                    av1[:],
                    AluOpType.add,
                )
            if r == N_ROUNDS - 1:
                # denominator complete for this nt: normalize + merge + store
                nc.vector.reciprocal(rh[:, nt : nt + 1], aug[:, nt, H : H + 1])
                nc.vector.tensor_scalar_mul(
                    rh[:, nt : nt + 1], rh[:, nt : nt + 1], 1.0 - MERGE
                )
                # feat already holds MERGE * features (host pre-scaled)
                o = out_pool.tile([128, H], F32, tag="out")
                nc.vector.scalar_tensor_tensor(
                    o[:],
                    aug[:, nt, 0:H],
                    rh[:, nt : nt + 1],
                    feat[:, nt, :],
                    op0=AluOpType.mult,
                    op1=AluOpType.add,
                )
                nc.sync.dma_start(d["out"][nt * 128 : (nt + 1) * 128, :], o[:])

    def emit_s_tiles(g, start, ntiles, tl_range, ets, rmax_t):
        csz = ntiles // len(mets)
        for tl in tl_range:
            met8 = mets[tl // csz]
            t = tl % csz
            pi, i = tl // 2, tl % 2
            if i == 0:
                ets.append(
                    et_pool.tile([128, 2, N_LOC], BF16, tag="et", name=f"et{g}_{pi}")
                )
            et = ets[pi]
            for nh in range(NH):
                sp = s_ps_pool.tile([128, 512], F32, tag="sps")
                for jc in range(HC):
                    nc.tensor.matmul(
                        sp[:],
                        met8[:, jc, t * 128 : (t + 1) * 128],
                        q2T[:, jc, nh * 512 : (nh + 1) * 512],
                        start=(jc == 0),
                        stop=(jc == HC - 1),
                    )
                nc.scalar.activation(
                    et[:, i, nh * 512 : (nh + 1) * 512],
                    sp[:],
                    mybir.ActivationFunctionType.Exp,
                    bias=bias_t[:],
                )
            # running elementwise max for this round's row-max
            if tl == 0:
                nc.vector.tensor_copy(rmax_t[:], et[:, 0, :])
            else:
                nc.vector.tensor_tensor(rmax_t[:], rmax_t[:], et[:, i, :], AluOpType.max)

    for g, (start, ntiles) in enumerate(ROUND_TILES):
        if g + 1 < N_ROUNDS:
            next_mets = load_met(*ROUND_TILES[g + 1])
            next_mv = load_mv(*ROUND_TILES[g + 1])
        if g == 3:
            # merge-side features (fp16, pre-scaled by MERGE) load late
            nc.sync.dma_start(
                feat[:], d["featm"].rearrange("(t p) h -> p t h", p=128)
            )
        r = g - 1  # round whose stats/AV are interleaved with S_g
        ets = []
        rmax_t = rmax_pool.tile([128, N_LOC], BF16, tag="rmax", name=f"rmax{g}")
        state[g] = {"rmax": rmax_t, "mv": mv_t, "ets": ets}

        # The whole stats->rescale chain sits at the TOP of the round: the
        # AV of round r-1 (end of the previous emission) absorbs round r's
        # exp/max tail, so transp8_r starts unstalled, and the rescale's
        # DVE mults run ahead of this round's max ops in the DVE queue --
        # the Pool casts then complete long before AV_r needs the pairs.
        # Round 1 has no preceding AV on the PE to absorb round 0's exp/max
        # tail, so lead with one S tile before the transposes there.
        lead = 1 if r == 0 else 0
        if lead:
            emit_s_tiles(g, start, ntiles, range(0, 1), ets, rmax_t)
        if r >= 0:
            emit_transposes(r)
            emit_stats_dve(r)
        emit_s_tiles(g, start, ntiles, range(lead, lead + 1), ets, rmax_t)
        if r >= 0:
            emit_frow_transposes(r)
            emit_frow_copy(r)
        emit_s_tiles(g, start, ntiles, range(lead + 1, lead + 2), ets, rmax_t)
        if r >= 0:
            emit_bcast(r)
            emit_rescale(r)
        emit_s_tiles(g, start, ntiles, range(lead + 2, ntiles), ets, rmax_t)
        if r >= 0:
            emit_av(r)
            del state[r]
        if g + 1 < N_ROUNDS:
            mets = next_mets
            mv_t = next_mv

    # epilogue: stats + AV for the last round
    r = N_ROUNDS - 1
    emit_transposes(r)
    emit_stats_dve(r)
    emit_frow_transposes(r)
    emit_frow_copy(r)
    emit_bcast(r)
    emit_rescale(r)
    emit_av(r)


def build_module():
    nc = bacc.Bacc("TRN2", target_bir_lowering=False, debug=False)
    d = {
        "featm": nc.dram_tensor("featm", [N_LOC, H], F16, kind="ExternalInput").ap(),
        "featT": nc.dram_tensor("featT", [H, N_LOC], F16, kind="ExternalInput").ap(),
        "memT": nc.dram_tensor("memT", [H, M], F16, kind="ExternalInput").ap(),
        "memv8": nc.dram_tensor("memv8", [M, VW8], F8, kind="ExternalInput").ap(),
        "wqh": nc.dram_tensor("wqh", [128, H // 128 * H], F16, kind="ExternalInput").ap(),
        "wkh": nc.dram_tensor("wkh", [128, H // 128 * H], F16, kind="ExternalInput").ap(),
        "bqh": nc.dram_tensor("bqh", [128, H // 128], F16, kind="ExternalInput").ap(),
        "out": nc.dram_tensor("out", [N_LOC, H], F32, kind="ExternalOutput").ap(),
    }
    with tile.TileContext(nc) as tc, ExitStack() as ctx:
        _emit(nc, tc, ctx, d)
    nc.compile()
    return nc


_CACHED = None


def _warrange(w):  # [512, 512] -> on-chip [p, c*h] layout, contiguous DMA
    w16 = np.asarray(w, dtype=np.float32).astype(np.float16)
    return np.ascontiguousarray(
        w16.reshape(H // 128, 128, H).transpose(1, 0, 2).reshape(128, -1)
    )


def _mem8(mem):  # [M, H] f32 -> [M, VW8] e4m3 of [32*V | 32 | pad]
    out = np.zeros((M, VW8), dtype=ml_dtypes.float8_e4m3)
    scaled = np.clip(mem * VSCALE, -240.0, 240.0)
    out[:, :H] = scaled.astype(ml_dtypes.float8_e4m3)
    out[:, H] = np.float32(VSCALE)
    return out


def kernel(features, memory_features, Wq, bq, Wk, bk=None, **_ignored):
    global _CACHED
    if _CACHED is None:
        _CACHED = build_module()
    nc = _CACHED

    features = np.ascontiguousarray(np.asarray(features, dtype=np.float32))
    memory_features = np.ascontiguousarray(np.asarray(memory_features, dtype=np.float32))
    memT = np.ascontiguousarray(memory_features.T.astype(np.float16))
    memv8 = _mem8(memory_features)
    feath = features.astype(np.float16)  # sharded then transposed per core
    featm = (MERGE * features).astype(np.float16)  # merge-side, pre-scaled
    wqh = _warrange(Wq)
    wkh = _warrange(Wk)
    bqh = np.ascontiguousarray(
        np.asarray(bq, dtype=np.float32).astype(np.float16).reshape(H // 128, 128).T
    )

    in_maps = []
    for c in range(N_CORES):
        in_maps.append(
            {
                "featm": featm[c * N_LOC : (c + 1) * N_LOC],
                "featT": np.ascontiguousarray(
                    feath[c * N_LOC : (c + 1) * N_LOC].T
                ),
                "memT": memT,
                "memv8": memv8,
                "wqh": wqh,
                "wkh": wkh,
                "bqh": bqh,
            }
        )
    res = run_bass_kernel_spmd(nc, in_maps, core_ids=list(range(N_CORES)))
    return np.concatenate([res.results[c]["out"] for c in range(N_CORES)], axis=0)
